# revision 1
# baseline (speedup 1.0000x reference)
"""FAGCN forward on 8 TRN2 NeuronCores (Bass/Tile).

Sharding: row-partition of nodes, 8 ways. The dense input projection
(h0 = relu(x @ t1^T + b)) is replicated on every core — cheaper than an
extra AllGather of h0. Per layer the edge phase gathers per-node "ext"
rows ([h bf16(256B) | b f32 | pad] = 512B) by column index with
dma_gather, then scatter-adds messages into 128-row PSUM windows via
one-hot matmuls on TensorE (the one-hot carries the edge weight
w = tanh(a[row]+b[col]+gb) * dinv[row]*dinv[col]). Per-edge a[row] is
recovered without any gather: a rank-1 broadcast of the window's a
values (PE) + masked row-reduce on DVE (tensor_tensor_reduce with the
same equality mask that forms the one-hot). Between the two FALayers the
owned ext rows are AllGathered. Head: transpose-by-matmul + t2 matmul +
log_softmax, all per window.
"""

import os
import sys
import numpy as np

sys.path.insert(0, "/opt/trn_rl_repo")

import concourse.bass as bass
import concourse.bacc as bacc
import concourse.mybir as mybir
import concourse.tile as tile
from concourse import library_config

F32 = mybir.dt.float32
BF16 = mybir.dt.bfloat16
I16 = mybir.dt.int16

# problem constants (self-contained per contract)
N_NODES = 50000
IN_CH = 256
HIDDEN = 128
OUT_CH = 64
EPS = 0.3
NCORES = 8
MAX_CALL = 2048
EXT_SLOTS = 128  # 512B ext row


def _install_profile_hook():
    import types
    name = "antenv.axon_hooks"
    if name in sys.modules:
        return
    try:
        import trn_agent_boot.trn_boot as tb
        hook = tb._ntff_profile_via_ctypes("/opt/axon/libaxon_pjrt.so")
    except Exception:
        hook = None
    mod = types.ModuleType(name)
    mod._hook = hook
    mod.get_axon_ntff_profile_hook = lambda: mod._hook
    mod.set_axon_ntff_profile_hook = lambda h: setattr(mod, "_hook", h)
    sys.modules[name] = mod


# ======================================================================
# Host preprocessing: common SPMD token-stream structure + per-core data
# ======================================================================

def preprocess(edge_index, n_nodes, ncores, lo_split):
    row = np.asarray(edge_index[0], dtype=np.int64)
    col = np.asarray(edge_index[1], dtype=np.int64)
    E = row.shape[0]
    r_per = n_nodes // ncores

    deg = np.bincount(row, minlength=n_nodes).astype(np.float64)
    dinv = np.where(deg > 0, 1.0 / np.sqrt(np.maximum(deg, 1.0)), 0.0)
    escale_all = (dinv[row] * dinv[col]).astype(np.float32)

    core = row // r_per
    lrow = row - core * r_per
    win = lrow // 128
    nwin = (r_per + 127) // 128
    is_hi = (col >= lo_split).astype(np.int64)

    order = np.lexsort((lrow, is_hi, win, core))
    core_s, win_s, hi_s = core[order], win[order], is_hi[order]
    lrow_s, col_s, esc_s = lrow[order], col[order], escale_all[order]

    key = (core_s * nwin + win_s) * 2 + hi_s
    cnt = np.bincount(key, minlength=ncores * nwin * 2).reshape(ncores, nwin, 2)
    sec_len = ((cnt.max(axis=0) + 127) // 128) * 128  # [nwin, 2]
    flat = sec_len.reshape(-1)
    starts = np.concatenate([[0], np.cumsum(flat)[:-1]]).astype(np.int64)
    sec_start = starts.reshape(nwin, 2)
    e_tok = int(flat.sum())

    col16 = np.zeros((ncores, e_tok), np.int16)
    rowrel = np.zeros((ncores, e_tok), np.float32)
    esc = np.zeros((ncores, e_tok), np.float32)

    grp_first = np.zeros(ncores * nwin * 2 + 1, np.int64)
    np.cumsum(cnt.reshape(-1), out=grp_first[1:])
    rank = np.arange(E) - grp_first[key]
    dest = sec_start[win_s, hi_s] + rank
    cval = np.where(hi_s == 1, col_s - lo_split, col_s).astype(np.int16)
    col16[core_s, dest] = cval
    rowrel[core_s, dest] = (lrow_s - win_s * 128).astype(np.float32)
    esc[core_s, dest] = esc_s

    pieces = []  # (tok_start, ntok, window, is_hi, first_in_win, last_in_win)
    for w in range(nwin):
        plist = []
        for h in range(2):
            s, ln = int(sec_start[w, h]), int(sec_len[w, h])
            off = 0
            while off < ln:
                n = min(MAX_CALL, ln - off)
                plist.append([s + off, n, w, h, False, False])
                off += n
        assert plist, f"window {w} empty"
        plist[0][4] = True
        plist[-1][5] = True
        pieces.extend(tuple(p) for p in plist)

    idx_dev = np.zeros((ncores, 128, e_tok // 16), np.int16)
    for (ts, nt, w, h, fw, lw) in pieces:
        blk = col16[:, ts:ts + nt].reshape(ncores, nt // 16, 16)
        blk = np.ascontiguousarray(np.transpose(blk, (0, 2, 1)))
        idx_dev[:, :, ts // 16:(ts + nt) // 16] = np.tile(blk, (1, 8, 1))
    rr_dev = np.ascontiguousarray(rowrel.reshape(ncores, -1, 128).transpose(0, 2, 1))
    es_dev = np.ascontiguousarray(esc.reshape(ncores, -1, 128).transpose(0, 2, 1))

    return {
        "pieces": pieces, "nwin": nwin, "e_tok": e_tok,
        "idx_dev": idx_dev, "rr_dev": rr_dev, "es_dev": es_dev,
    }


# ======================================================================
# Kernel builder
# ======================================================================

def build_kernel(meta, n_nodes, in_ch, hidden, out_ch, eps, lo_split, ncores):
    nwin = meta["nwin"]
    e_tok = meta["e_tok"]
    pieces = meta["pieces"]
    nchunk_tot = (n_nodes + 127) // 128
    last_chunk_rows = n_nodes - 128 * (nchunk_tot - 1)
    r_per = n_nodes // ncores
    last_win_rows = r_per - 128 * (nwin - 1)
    kt = in_ch // 128
    hh = hidden // 2  # f32 slots holding bf16 h

    nc = bacc.Bacc("TRN2", target_bir_lowering=False, debug=False,
                   num_devices=ncores, num_swdge_queues=int(os.environ.get("KNQ", "4")))

    # ---- I/O ----
    xt = nc.dram_tensor("xt", [nchunk_tot, in_ch + 1, 128], BF16, kind="ExternalInput")
    xt_own = nc.dram_tensor("xt_own", [nwin, in_ch + 1, 128], BF16, kind="ExternalInput")
    t1wt = nc.dram_tensor("t1wt", [in_ch + 1, hidden], BF16, kind="ExternalInput")
    gwrep = nc.dram_tensor("gwrep", [4, 128, hidden], BF16, kind="ExternalInput")
    gbrep = nc.dram_tensor("gbrep", [128, 2], F32, kind="ExternalInput")
    t2wt = nc.dram_tensor("t2wt", [hidden, out_ch], F32, kind="ExternalInput")
    t2b = nc.dram_tensor("t2b", [1, out_ch], F32, kind="ExternalInput")
    iota_in = nc.dram_tensor("iota", [128, 128], F32, kind="ExternalInput")
    ident_in = nc.dram_tensor("ident", [128, 128], F32, kind="ExternalInput")
    ones_in = nc.dram_tensor("ones", [1, 128], F32, kind="ExternalInput")
    idx_in = nc.dram_tensor("idx", [128, e_tok // 16], I16, kind="ExternalInput")
    rr_in = nc.dram_tensor("rr", [128, e_tok // 128], F32, kind="ExternalInput")
    es_in = nc.dram_tensor("es", [128, e_tok // 128], F32, kind="ExternalInput")
    out = nc.dram_tensor("out", [r_per, out_ch], F32, kind="ExternalOutput")

    ext0 = nc.dram_tensor("ext0", [nchunk_tot * 128, EXT_SLOTS], F32)
    agi = nc.dram_tensor("agi", [r_per, EXT_SLOTS], F32)
    ago = nc.dram_tensor("ago", [r_per * ncores, EXT_SLOTS], F32)

    with tile.TileContext(nc) as tc:
        nc.gpsimd.load_library(library_config.mlp)
        with tc.tile_pool(name="consts", bufs=1) as cp:
            t1wt_sb = cp.tile([128, kt, hidden], BF16, tag="t1wt")
            nc.sync.dma_start(t1wt_sb[:], bass.AP(t1wt, 0, [[hidden, 128], [128 * hidden, kt], [1, hidden]]))
            t1b_sb = cp.tile([1, hidden], BF16, tag="t1b")
            nc.sync.dma_start(t1b_sb[:], t1wt.ap()[in_ch:in_ch + 1, :])
            gw_sb = cp.tile([128, 4, hidden], BF16, tag="gw")
            nc.sync.dma_start(gw_sb[:], bass.AP(gwrep, 0, [[hidden, 128], [128 * hidden, 4], [1, hidden]]))
            gb_sb = cp.tile([128, 2], F32, tag="gb")
            nc.sync.dma_start(gb_sb[:], gbrep.ap())
            t2wt_sb = cp.tile([128, out_ch], F32, tag="t2wt")
            nc.sync.dma_start(t2wt_sb[:], t2wt.ap())
            t2b_sb = cp.tile([1, out_ch], F32, tag="t2b")
            nc.sync.dma_start(t2b_sb[:], t2b.ap())
            iota_sb = cp.tile([128, 128], F32, tag="iota")
            nc.sync.dma_start(iota_sb[:], iota_in.ap())
            ident_sb = cp.tile([128, 128], F32, tag="ident")
            nc.sync.dma_start(ident_sb[:], ident_in.ap())
            ones_sb = cp.tile([1, 128], F32, tag="ones")
            nc.sync.dma_start(ones_sb[:], ones_in.ap())

            rawsc = cp.tile([128, nwin, hidden], F32, tag="rawsc")
            a_arr = cp.tile([128, nwin, 2], F32, tag="a_arr")
            idxt = cp.tile([128, e_tok // 16], I16, tag="idxt")
            nc.sync.dma_start(idxt[:], idx_in.ap())
            rr_sb = cp.tile([128, e_tok // 128], F32, tag="rr")
            nc.sync.dma_start(rr_sb[:], rr_in.ap())
            es_sb = cp.tile([128, e_tok // 128], F32, tag="es")
            nc.sync.dma_start(es_sb[:], es_in.ap())

            # ------------- L0 prep: replicated ext0 over all nodes -------------
            with tc.tile_pool(name="prep", bufs=4) as pp, \
                 tc.tile_pool(name="prep_ps", bufs=4, space="PSUM") as pps:
                for ch in range(nchunk_tot):
                    xt_sb = pp.tile([128, kt, 128], BF16, tag="xt")
                    nc.sync.dma_start(xt_sb[:], xt.ap()[ch, 0:in_ch, :].rearrange("(k p) r -> p k r", p=128))
                    xb_sb = pp.tile([1, 128], BF16, tag="xb")
                    nc.sync.dma_start(xb_sb[:], xt.ap()[ch, in_ch:in_ch + 1, :])
                    ps = pps.tile([128, hidden], F32, tag="h0ps")
                    for k in range(kt):
                        nc.tensor.matmul(ps[:], xt_sb[:, k, :], t1wt_sb[:, k, :],
                                         start=(k == 0), stop=False)
                    nc.tensor.matmul(ps[:], xb_sb[:], t1b_sb[:], start=False, stop=True)
                    ext_sb = pp.tile([128, EXT_SLOTS], F32, tag="ext")
                    nc.vector.memset(ext_sb[:, hh:], 0.0)
                    hbf = ext_sb[:, 0:hh].bitcast(BF16)
                    nc.scalar.activation(hbf, ps[:], mybir.ActivationFunctionType.Relu)
                    scr = pp.tile([128, hidden], F32, tag="scr")
                    nc.vector.tensor_tensor(out=scr[:], in0=hbf, in1=gw_sb[:, 1, :],
                                            op=mybir.AluOpType.mult)
                    nc.vector.reduce_sum(out=ext_sb[:, hh:hh + 1], in_=scr[:],
                                         axis=mybir.AxisListType.X)
                    nc.sync.dma_start(ext0.ap()[ch * 128:(ch + 1) * 128, :], ext_sb[:])

                # own rows: rawsc + a1
                for w in range(nwin):
                    xt_sb = pp.tile([128, kt, 128], BF16, tag="xt")
                    nc.sync.dma_start(xt_sb[:], xt_own.ap()[w, 0:in_ch, :].rearrange("(k p) r -> p k r", p=128))
                    xb_sb = pp.tile([1, 128], BF16, tag="xb")
                    nc.sync.dma_start(xb_sb[:], xt_own.ap()[w, in_ch:in_ch + 1, :])
                    ps = pps.tile([128, hidden], F32, tag="h0ps")
                    for k in range(kt):
                        nc.tensor.matmul(ps[:], xt_sb[:, k, :], t1wt_sb[:, k, :],
                                         start=(k == 0), stop=False)
                    nc.tensor.matmul(ps[:], xb_sb[:], t1b_sb[:], start=False, stop=True)
                    nc.vector.tensor_scalar(out=rawsc[:, w, :], in0=ps[:],
                                            scalar1=0.0, scalar2=eps,
                                            op0=mybir.AluOpType.max,
                                            op1=mybir.AluOpType.mult)
                    hb = pp.tile([128, hidden], BF16, tag="hb")
                    nc.scalar.activation(hb[:], ps[:], mybir.ActivationFunctionType.Relu)
                    scr = pp.tile([128, hidden], F32, tag="scr")
                    nc.vector.tensor_tensor(out=scr[:], in0=hb[:], in1=gw_sb[:, 0, :],
                                            op=mybir.AluOpType.mult)
                    tmp1 = pp.tile([128, 1], F32, tag="tmp1")
                    nc.vector.reduce_sum(out=tmp1[:], in_=scr[:],
                                         axis=mybir.AxisListType.X)
                    nc.vector.tensor_scalar(out=a_arr[:, w, 0:1], in0=tmp1[:],
                                            scalar1=gb_sb[:, 0:1], scalar2=None,
                                            op0=mybir.AluOpType.add)

            # ------------- edge phase (both layers) -------------
            def emit_layer(l, table):
                lo_ap = table.ap()
                hi_ap = table.ap()[lo_split:, :]
                with tc.tile_pool(name=f"g{l}", bufs=4) as gp, \
                     tc.tile_pool(name=f"eq{l}", bufs=20) as eqp, \
                     tc.tile_pool(name=f"s{l}", bufs=3) as sp, \
                     tc.tile_pool(name=f"oh{l}", bufs=4) as ohp, \
                     tc.tile_pool(name=f"fin{l}", bufs=2) as fp, \
                     tc.tile_pool(name=f"psA{l}", bufs=1, space="PSUM") as psA, \
                     tc.tile_pool(name=f"psW{l}", bufs=2, space="PSUM") as psW, \
                     tc.tile_pool(name=f"psH{l}", bufs=1, space="PSUM") as psH:
                    W_ps = None
                    qi = 0
                    for (ts, nt, w, hi, first, last) in pieces:
                        if first:
                            # window setup: A broadcast of a-values
                            awt_ps = psA.tile([1, 128], F32, tag="awt")
                            nc.tensor.matmul(awt_ps[:], a_arr[:, w, l:l + 1], ident_sb[:],
                                             start=True, stop=True)
                            awt_sb = sp.tile([1, 128], F32, tag="awt_sb")
                            nc.vector.tensor_copy(awt_sb[:], awt_ps[:])
                            A_ps = psA.tile([128, 128], F32, tag="A")
                            nc.tensor.matmul(A_ps[:], ones_sb[:], awt_sb[:],
                                             start=True, stop=True)
                            W_ps = psW.tile([128, hidden], F32, tag="W")
                        ntile = nt // 128
                        G = gp.tile([128, MAX_CALL // 128, EXT_SLOTS], F32, tag="G")
                        nc.gpsimd.dma_gather(
                            out_ap=G[:, 0:ntile, :],
                            in_ap=(hi_ap if hi else lo_ap),
                            idxs_ap=idxt[:, ts // 16:(ts + nt) // 16],
                            num_idxs=nt, num_idxs_reg=nt, elem_size=EXT_SLOTS,
                            single_packet=False, queue_num=qi % int(os.environ.get("KNQ", "4")))
                        qi += 1
                        c0 = ts // 128
                        atok = sp.tile([128, MAX_CALL // 128], F32, tag="atok")
                        eqs = []
                        for t in range(ntile):
                            eq = eqp.tile([128, 128], F32, tag="eq")
                            nc.vector.tensor_scalar(out=eq[:], in0=iota_sb[:],
                                                    scalar1=rr_sb[:, c0 + t:c0 + t + 1],
                                                    scalar2=None,
                                                    op0=mybir.AluOpType.is_equal)
                            scr2 = sp.tile([128, 128], F32, tag="scr2")
                            nc.vector.tensor_tensor(out=scr2[:], in0=A_ps[:], in1=eq[:],
                                                    op=mybir.AluOpType.mult)
                            nc.vector.reduce_sum(out=atok[:, t:t + 1], in_=scr2[:],
                                                 axis=mybir.AxisListType.X)
                            eqs.append(eq)
                        arg = sp.tile([128, MAX_CALL // 128], F32, tag="arg")
                        nc.vector.tensor_tensor(out=arg[:, 0:ntile], in0=atok[:, 0:ntile],
                                                in1=G[:, 0:ntile, hh],
                                                op=mybir.AluOpType.add)
                        gt = sp.tile([128, MAX_CALL // 128], F32, tag="gt")
                        nc.scalar.activation(gt[:, 0:ntile], arg[:, 0:ntile],
                                             mybir.ActivationFunctionType.Tanh)
                        wt = sp.tile([128, MAX_CALL // 128], F32, tag="wt")
                        nc.vector.tensor_tensor(out=wt[:, 0:ntile], in0=gt[:, 0:ntile],
                                                in1=es_sb[:, c0:c0 + ntile],
                                                op=mybir.AluOpType.mult)
                        for t in range(ntile):
                            oh = ohp.tile([128, 128], BF16, tag="oh")
                            nc.vector.tensor_scalar(out=oh[:], in0=eqs[t][:],
                                                    scalar1=wt[:, t:t + 1], scalar2=None,
                                                    op0=mybir.AluOpType.mult)
                            nc.tensor.matmul(W_ps[:], oh[:], G[:, t, 0:hh].bitcast(BF16),
                                             start=(first and t == 0),
                                             stop=(last and t == ntile - 1))
                        if last:
                            rows = 128 if w < nwin - 1 else last_win_rows
                            h_sb = fp.tile([128, hidden], F32, tag="h")
                            nc.vector.tensor_tensor(out=h_sb[:], in0=W_ps[:],
                                                    in1=rawsc[:, w, :],
                                                    op=mybir.AluOpType.add)
                            if l == 0:
                                ext1 = fp.tile([128, EXT_SLOTS], F32, tag="ext1")
                                nc.vector.memset(ext1[:, hh:], 0.0)
                                h1b = ext1[:, 0:hh].bitcast(BF16)
                                nc.vector.tensor_copy(h1b, h_sb[:])
                                scr3 = fp.tile([128, hidden], F32, tag="scr3")
                                nc.vector.tensor_tensor(out=scr3[:], in0=h1b,
                                                        in1=gw_sb[:, 3, :],
                                                        op=mybir.AluOpType.mult)
                                nc.vector.reduce_sum(out=ext1[:, hh:hh + 1], in_=scr3[:],
                                                     axis=mybir.AxisListType.X)
                                nc.vector.tensor_tensor(out=scr3[:], in0=h1b,
                                                        in1=gw_sb[:, 2, :],
                                                        op=mybir.AluOpType.mult)
                                tmp2 = fp.tile([128, 1], F32, tag="tmp2")
                                nc.vector.reduce_sum(out=tmp2[:], in_=scr3[:],
                                                     axis=mybir.AxisListType.X)
                                nc.vector.tensor_scalar(out=a_arr[:, w, 1:2], in0=tmp2[:],
                                                        scalar1=gb_sb[:, 1:2], scalar2=None,
                                                        op0=mybir.AluOpType.add)
                                nc.sync.dma_start(agi.ap()[w * 128:w * 128 + rows, :],
                                                  ext1[0:rows, :])
                            else:
                                ht_ps = psH.tile([128, 128], F32, tag="ht")
                                nc.tensor.matmul(ht_ps[:], h_sb[:], ident_sb[:],
                                                 start=True, stop=True)
                                ht_sb = fp.tile([128, 128], F32, tag="ht_sb")
                                nc.vector.tensor_copy(ht_sb[:], ht_ps[:])
                                o_ps = psH.tile([128, out_ch], F32, tag="ops")
                                nc.tensor.matmul(o_ps[:], ht_sb[:], t2wt_sb[:],
                                                 start=True, stop=False)
                                nc.tensor.matmul(o_ps[:], ones_sb[:], t2b_sb[:],
                                                 start=False, stop=True)
                                nm = fp.tile([128, 1], F32, tag="nm")
                                nc.vector.reduce_max(out=nm[:], in_=o_ps[:],
                                                     axis=mybir.AxisListType.X,
                                                     negate=True)
                                e_sb = fp.tile([128, out_ch], F32, tag="e")
                                nc.scalar.activation(e_sb[:], o_ps[:],
                                                     mybir.ActivationFunctionType.Exp,
                                                     bias=nm[:])
                                s_sb = fp.tile([128, 1], F32, tag="s")
                                nc.vector.reduce_sum(out=s_sb[:], in_=e_sb[:],
                                                     axis=mybir.AxisListType.X)
                                ls = fp.tile([128, 1], F32, tag="ls")
                                nc.scalar.activation(ls[:], s_sb[:],
                                                     mybir.ActivationFunctionType.Ln)
                                o_sb = fp.tile([128, out_ch], F32, tag="o")
                                nc.vector.tensor_scalar(out=o_sb[:], in0=o_ps[:],
                                                        scalar1=nm[:], scalar2=ls[:],
                                                        op0=mybir.AluOpType.add,
                                                        op1=mybir.AluOpType.subtract)
                                nc.sync.dma_start(out.ap()[w * 128:w * 128 + rows, :],
                                                  o_sb[0:rows, :])

            if not os.environ.get("KSKIPL1"):
                emit_layer(0, ext0)
            else:
                ag_z = cp.tile([128, EXT_SLOTS], F32, tag="agz")
                nc.vector.memset(ag_z[:], 0.0)
                for w in range(nwin):
                    rows = 128 if w < nwin - 1 else last_win_rows
                    nc.sync.dma_start(agi.ap()[w * 128:w * 128 + rows, :], ag_z[0:rows, :])
                nc.vector.memset(a_arr[:, :, 1:2], 0.0)
            if os.environ.get("KSKIPCC"):
                for c in range(ncores):
                    nc.sync.dma_start(ago.ap()[c * r_per:(c + 1) * r_per, :], agi.ap())
            else:
                nc.gpsimd.collective_compute(
                    "AllGather", mybir.AluOpType.bypass,
                    replica_groups=[list(range(ncores))],
                    ins=[agi.ap().opt()], outs=[ago.ap().opt()])
            if not os.environ.get("KSKIPL2"):
                emit_layer(1, ago)
            else:
                o_z = cp.tile([128, out_ch], F32, tag="oz")
                nc.vector.memset(o_z[:], 0.0)
                for w in range(nwin):
                    rows = 128 if w < nwin - 1 else last_win_rows
                    nc.sync.dma_start(out.ap()[w * 128:w * 128 + rows, :], o_z[0:rows, :])

    return nc


# ======================================================================
# Host driver
# ======================================================================

def _bf16(a):
    import ml_dtypes
    return np.asarray(a, dtype=ml_dtypes.bfloat16)


def kernel_run(x, edge_index, t1_w, t1_b, gate_w, gate_b, t2_w, t2_b,
               n_nodes=N_NODES, in_ch=IN_CH, hidden=HIDDEN, out_ch=OUT_CH,
               eps=EPS, ncores=NCORES, lo_split=None, trace=False):
    _install_profile_hook()
    from concourse import bass_utils

    if lo_split is None:
        lo_split = min(25000, ((n_nodes + 1) // 2 + 127) // 128 * 128)
    meta = preprocess(edge_index, n_nodes, ncores, lo_split)
    nwin = meta["nwin"]
    r_per = n_nodes // ncores
    nchunk_tot = (n_nodes + 127) // 128

    nc = build_kernel(meta, n_nodes, in_ch, hidden, out_ch, eps, lo_split, ncores)
    nc.finalize()

    # host arrays
    x = np.asarray(x, np.float32)
    xT = np.concatenate([x.T, np.ones((1, x.shape[0]), np.float32)], axis=0)  # [in+1, N]
    pad_n = nchunk_tot * 128
    xT_pad = np.zeros((in_ch + 1, pad_n), np.float32)
    xT_pad[:, :n_nodes] = xT
    xt_tiled = _bf16(np.ascontiguousarray(
        xT_pad.reshape(in_ch + 1, nchunk_tot, 128).transpose(1, 0, 2)))
    t1wt_h = _bf16(np.concatenate([np.asarray(t1_w, np.float32).T,
                                   np.asarray(t1_b, np.float32)[None, :]], axis=0))
    gw = np.asarray(gate_w, np.float32)
    gwrep_h = _bf16(np.stack([
        np.tile(gw[0, :hidden][None, :], (128, 1)),
        np.tile(gw[0, hidden:][None, :], (128, 1)),
        np.tile(gw[1, :hidden][None, :], (128, 1)),
        np.tile(gw[1, hidden:][None, :], (128, 1))]))
    gbrep_h = np.tile(np.asarray(gate_b, np.float32)[None, :], (128, 1))
    t2wt_h = np.ascontiguousarray(np.asarray(t2_w, np.float32).T)
    t2b_h = np.asarray(t2_b, np.float32)[None, :]
    iota_h = np.tile(np.arange(128, dtype=np.float32)[None, :], (128, 1))
    ident_h = np.eye(128, dtype=np.float32)
    ones_h = np.ones((1, 128), np.float32)

    pad_own = nwin * 128
    in_maps = []
    for c in range(ncores):
        sl = np.zeros((in_ch + 1, pad_own), np.float32)
        take = min(pad_own, xT.shape[1] - c * r_per)
        sl[:, :take] = xT[:, c * r_per: c * r_per + take]
        xt_own_h = _bf16(np.ascontiguousarray(
            sl.reshape(in_ch + 1, nwin, 128).transpose(1, 0, 2)))
        in_maps.append({
            "xt": xt_tiled, "xt_own": xt_own_h, "t1wt": t1wt_h,
            "gwrep": gwrep_h, "gbrep": gbrep_h, "t2wt": t2wt_h, "t2b": t2b_h,
            "iota": iota_h, "ident": ident_h, "ones": ones_h,
            "idx": meta["idx_dev"][c], "rr": meta["rr_dev"][c],
            "es": meta["es_dev"][c],
        })

    res = bass_utils.run_bass_kernel_spmd(
        nc, in_maps, core_ids=list(range(ncores)), trace=trace)
    outp = np.concatenate([res.results[c]["out"] for c in range(ncores)], axis=0)
    return outp[:n_nodes], res


def kernel(**inputs):
    x = inputs["x"]
    edge_index = inputs["edge_index"]
    outp, _ = kernel_run(
        x, edge_index, inputs["t1_w"], inputs["t1_b"], inputs["gate_w"],
        inputs["gate_b"], inputs["t2_w"], inputs["t2_b"])
    return np.asarray(outp, np.float32)



# revision 13
# speedup vs baseline: 1.1953x; 1.1953x over previous
"""FAGCN forward on 8 TRN2 NeuronCores (Bass/Tile) — v2.

Sharding: row-partition of nodes, 8 ways. The dense input projection
(h = relu(x @ t1^T + b)) is replicated on every core into a 512B-stride
gather table [h bf16 x128 | b f32 | pad]. Per layer the edge phase is a
two-stream token walk ([all-lo windows][all-hi windows], int16 gather
indices split at lo_split): big dma_gather calls (6144 edges) fetch
source rows; per 128-edge tile a bf16 one-hot (4x DVE mode) both
recovers a[row] (tensor_tensor_reduce against a partition-replicated
a-broadcast) and scatter-adds w*h[col] into a per-window PSUM
accumulator via TensorE. Window results accumulate in SBUF across the
two streams. Between layers the owned rows are AllGathered. The head
(t2 matmul + log_softmax) runs as a final pass so the activation table
is not thrashed.
"""

import os
import sys
import numpy as np

sys.path.insert(0, "/opt/trn_rl_repo")

import concourse.bass as bass
import concourse.bacc as bacc
import concourse.mybir as mybir
import concourse.tile as tile
from concourse import library_config

F32 = mybir.dt.float32
BF16 = mybir.dt.bfloat16
I16 = mybir.dt.int16

# problem constants (self-contained per contract)
N_NODES = 50000
IN_CH = 256
HIDDEN = 128
OUT_CH = 64
EPS = 0.3
NCORES = 8
CALL_TOKENS = int(os.environ.get("KCT", "2048"))
CT_MAX = CALL_TOKENS // 128
EXT_SLOTS = 128   # 512B gather record
B_SLOT = 64       # f32 slot holding the gate b-term
PREP_GRP = 8


def _install_profile_hook():
    import types
    name = "antenv.axon_hooks"
    if name in sys.modules:
        return
    try:
        import trn_agent_boot.trn_boot as tb
        hook = tb._ntff_profile_via_ctypes("/opt/axon/libaxon_pjrt.so")
    except Exception:
        hook = None
    mod = types.ModuleType(name)
    mod._hook = hook
    mod.get_axon_ntff_profile_hook = lambda: mod._hook
    mod.set_axon_ntff_profile_hook = lambda h: setattr(mod, "_hook", h)
    sys.modules[name] = mod


# ======================================================================
# Host preprocessing: SPMD token streams + per-core data
# ======================================================================

def preprocess(edge_index, n_nodes, ncores, lo_split):
    row = np.asarray(edge_index[0], dtype=np.int64)
    col = np.asarray(edge_index[1], dtype=np.int64)
    E = row.shape[0]
    r_per = n_nodes // ncores
    nwin = (r_per + 127) // 128

    deg = np.bincount(row, minlength=n_nodes).astype(np.float64)
    dinv = np.where(deg > 0, 1.0 / np.sqrt(np.maximum(deg, 1.0)), 0.0)
    escale_all = (dinv[row] * dinv[col]).astype(np.float32)

    core = row // r_per
    lrow = row - core * r_per
    win = lrow // 128
    is_hi = (col >= lo_split).astype(np.int64)

    # stream order: core, then stream (lo/hi), then window, then lrow
    order = np.lexsort((lrow, win, is_hi, core))
    core_s, win_s, hi_s = core[order], win[order], is_hi[order]
    lrow_s, col_s, esc_s = lrow[order], col[order], escale_all[order]

    key = (core_s * 2 + hi_s) * nwin + win_s
    cnt = np.bincount(key, minlength=ncores * 2 * nwin).reshape(ncores, 2, nwin)
    sec_len = ((cnt.max(axis=0) + 127) // 128) * 128  # [2, nwin]
    L_lo = int(sec_len[0].sum())
    L_hi = int(sec_len[1].sum())
    e_tok = L_lo + L_hi
    sec_start = np.zeros((2, nwin), np.int64)
    sec_start[0] = np.concatenate([[0], np.cumsum(sec_len[0])[:-1]])
    sec_start[1] = L_lo + np.concatenate([[0], np.cumsum(sec_len[1])[:-1]])

    col16 = np.zeros((ncores, e_tok), np.int16)
    rowrel = np.zeros((ncores, e_tok), np.float32)
    esc = np.zeros((ncores, e_tok), np.float32)

    grp_first = np.zeros(ncores * 2 * nwin + 1, np.int64)
    np.cumsum(cnt.reshape(-1), out=grp_first[1:])
    rank = np.arange(E) - grp_first[key]
    dest = sec_start[hi_s, win_s] + rank
    cval = np.where(hi_s == 1, col_s - lo_split, col_s).astype(np.int16)
    col16[core_s, dest] = cval
    rowrel[core_s, dest] = (lrow_s - win_s * 128).astype(np.float32)
    esc[core_s, dest] = esc_s

    # gather calls per stream
    calls = []  # (stream, ts, nt)
    for h, base, L in ((0, 0, L_lo), (1, L_lo, L_hi)):
        off = 0
        while off < L:
            nt = min(CALL_TOKENS, L - off)
            calls.append((h, base + off, nt))
            off += nt

    idx_dev = np.zeros((ncores, 128, e_tok // 16), np.int16)
    for (h, ts, nt) in calls:
        blk = col16[:, ts:ts + nt].reshape(ncores, nt // 16, 16)
        blk = np.ascontiguousarray(np.transpose(blk, (0, 2, 1)))
        idx_dev[:, :, ts // 16:(ts + nt) // 16] = np.tile(blk, (1, 8, 1))
    rr_dev = np.ascontiguousarray(rowrel.reshape(ncores, -1, 128).transpose(0, 2, 1))
    es_dev = np.ascontiguousarray(esc.reshape(ncores, -1, 128).transpose(0, 2, 1))

    return {
        "nwin": nwin, "e_tok": e_tok, "sec_len": sec_len, "calls": calls,
        "idx_dev": idx_dev, "rr_dev": rr_dev, "es_dev": es_dev,
    }


# ======================================================================
# Kernel builder
# ======================================================================

def build_kernel(meta, n_nodes, in_ch, hidden, out_ch, eps, lo_split, ncores):
    nwin = meta["nwin"]
    e_tok = meta["e_tok"]
    sec_len = meta["sec_len"]
    calls = meta["calls"]
    nchunk_tot = (n_nodes + 127) // 128
    r_per = n_nodes // ncores
    last_win_rows = r_per - 128 * (nwin - 1)
    kt = in_ch // 128
    hh = hidden // 2  # f32 slots holding the bf16 h vector

    # tile -> window map, and burst boundaries per (stream, window)
    tiles_w = []
    burst = {}  # (h, w) -> (gfirst, glast) in global tile idx
    for h in range(2):
        for w in range(nwin):
            ntl = int(sec_len[h, w]) // 128
            if ntl == 0:
                continue
            g0 = len(tiles_w)
            tiles_w.extend([w] * ntl)
            burst[(h, w)] = (g0, g0 + ntl - 1)
    assert len(tiles_w) == e_tok // 128
    last_stream = {}
    for w in range(nwin):
        last_stream[w] = 1 if (1, w) in burst else 0

    ngrp = nchunk_tot // PREP_GRP
    grp_rem = nchunk_tot - ngrp * PREP_GRP
    ogrp = nwin // PREP_GRP
    ogrp_rem = nwin - ogrp * PREP_GRP

    nc = bacc.Bacc("TRN2", target_bir_lowering=False, debug=False,
                   num_devices=ncores, num_swdge_queues=4)

    # ---- I/O ----
    # xtg: host-prearranged [group, 128p, grp*kt, 128] bf16 (+ ones row separately)
    xtg = nc.dram_tensor("xtg", [ngrp + (1 if grp_rem else 0), 128, PREP_GRP * kt, 128], BF16, kind="ExternalInput")
    xbg = nc.dram_tensor("xbg", [ngrp + (1 if grp_rem else 0), 1, PREP_GRP, 128], BF16, kind="ExternalInput")
    xtog = nc.dram_tensor("xtog", [ogrp + (1 if ogrp_rem else 0), 128, PREP_GRP * kt, 128], BF16, kind="ExternalInput")
    xbog = nc.dram_tensor("xbog", [ogrp + (1 if ogrp_rem else 0), 1, PREP_GRP, 128], BF16, kind="ExternalInput")
    t1wt = nc.dram_tensor("t1wt", [in_ch + 1, hidden], BF16, kind="ExternalInput")
    gwrep = nc.dram_tensor("gwrep", [4, 128, hidden], BF16, kind="ExternalInput")
    gbrep = nc.dram_tensor("gbrep", [128, 2], F32, kind="ExternalInput")
    t2wt = nc.dram_tensor("t2wt", [hidden, out_ch], F32, kind="ExternalInput")
    t2b = nc.dram_tensor("t2b", [1, out_ch], F32, kind="ExternalInput")
    iota_in = nc.dram_tensor("iota", [128, 128], BF16, kind="ExternalInput")
    identb_in = nc.dram_tensor("identb", [128, 128], BF16, kind="ExternalInput")
    ident_in = nc.dram_tensor("ident", [128, 128], F32, kind="ExternalInput")
    onesb_in = nc.dram_tensor("onesb", [1, 128], BF16, kind="ExternalInput")
    ones_in = nc.dram_tensor("ones", [1, 128], F32, kind="ExternalInput")
    idx_in = nc.dram_tensor("idx", [128, e_tok // 16], I16, kind="ExternalInput")
    rr_in = nc.dram_tensor("rr", [128, e_tok // 128], F32, kind="ExternalInput")
    es_in = nc.dram_tensor("es", [128, e_tok // 128], F32, kind="ExternalInput")
    out = nc.dram_tensor("out", [r_per, out_ch], F32, kind="ExternalOutput")

    ext0 = nc.dram_tensor("ext0", [nchunk_tot * 128, EXT_SLOTS], F32)
    agi = nc.dram_tensor("agi", [r_per, EXT_SLOTS], F32)
    ago = nc.dram_tensor("ago", [r_per * ncores, EXT_SLOTS], F32)

    with tile.TileContext(nc) as tc:
        nc.gpsimd.load_library(library_config.mlp)
        with tc.tile_pool(name="consts", bufs=1) as cp:
            t1wt_sb = cp.tile([128, kt, hidden], BF16, tag="t1wt")
            nc.sync.dma_start(t1wt_sb[:], bass.AP(t1wt, 0, [[hidden, 128], [128 * hidden, kt], [1, hidden]]))
            t1b_sb = cp.tile([1, hidden], BF16, tag="t1b")
            nc.sync.dma_start(t1b_sb[:], t1wt.ap()[in_ch:in_ch + 1, :])
            gw_sb = cp.tile([128, 4, hidden], BF16, tag="gw")
            nc.sync.dma_start(gw_sb[:], bass.AP(gwrep, 0, [[hidden, 128], [128 * hidden, 4], [1, hidden]]))
            gb_sb = cp.tile([128, 2], F32, tag="gb")
            nc.sync.dma_start(gb_sb[:], gbrep.ap())
            t2wt_sb = cp.tile([128, out_ch], F32, tag="t2wt")
            nc.sync.dma_start(t2wt_sb[:], t2wt.ap())
            t2b_sb = cp.tile([1, out_ch], F32, tag="t2b")
            nc.sync.dma_start(t2b_sb[:], t2b.ap())
            iota_sb = cp.tile([128, 128], BF16, tag="iota")
            nc.sync.dma_start(iota_sb[:], iota_in.ap())
            identb_sb = cp.tile([128, 128], BF16, tag="identb")
            nc.sync.dma_start(identb_sb[:], identb_in.ap())
            ident_sb = cp.tile([128, 128], F32, tag="ident")
            nc.sync.dma_start(ident_sb[:], ident_in.ap())
            onesb_sb = cp.tile([1, 128], BF16, tag="onesb")
            nc.sync.dma_start(onesb_sb[:], onesb_in.ap())
            ones_sb = cp.tile([1, 128], F32, tag="ones")
            nc.sync.dma_start(ones_sb[:], ones_in.ap())
            idxt = cp.tile([128, e_tok // 16], I16, tag="idxt")
            nc.sync.dma_start(idxt[:], idx_in.ap())
            rr_sb = cp.tile([128, e_tok // 128], F32, tag="rr")
            nc.sync.dma_start(rr_sb[:], rr_in.ap())
            es_sb = cp.tile([128, e_tok // 128], F32, tag="es")
            nc.sync.dma_start(es_sb[:], es_in.ap())

            rawsc = cp.tile([128, nwin, hidden], F32, tag="rawsc")
            acc = cp.tile([128, nwin, hidden], F32, tag="acc")
            a_arr = cp.tile([128, nwin, 2], BF16, tag="a_arr")
            A_all = cp.tile([128, nwin, 128], BF16, tag="A_all")

            # ---------------- prep: replicated gather table ----------------
            with tc.tile_pool(name="prep", bufs=3) as pp, \
                 tc.tile_pool(name="prep_s", bufs=6) as pscr, \
                 tc.tile_pool(name="prep_ps", bufs=4, space="PSUM") as pps:

                def prep_group(gi, gcnt, xt_t, xb_t, own):
                    xt_sb = pp.tile([128, PREP_GRP * kt, 128], BF16, tag="xt")
                    nc.sync.dma_start(xt_sb[:, 0:gcnt * kt, :], xt_t.ap()[gi, :, 0:gcnt * kt, :])
                    xb_sb = pp.tile([1, PREP_GRP, 128], BF16, tag="xb")
                    nc.sync.dma_start(xb_sb[:, 0:gcnt, :], xb_t.ap()[gi, :, 0:gcnt, :])
                    if not own:
                        extg = pp.tile([128, PREP_GRP, B_SLOT + 1], F32, tag="extg")
                    for c in range(gcnt):
                        ps = pps.tile([128, hidden], F32, tag="h0ps")
                        for k in range(kt):
                            nc.tensor.matmul(ps[:], xt_sb[:, c * kt + k, :], t1wt_sb[:, k, :],
                                             start=(k == 0), stop=False)
                        nc.tensor.matmul(ps[:], xb_sb[:, c, :], t1b_sb[:], start=False, stop=True)
                        if own:
                            w = gi * PREP_GRP + c
                            nc.vector.tensor_scalar(out=rawsc[:, w, :], in0=ps[:],
                                                    scalar1=0.0, scalar2=eps,
                                                    op0=mybir.AluOpType.max,
                                                    op1=mybir.AluOpType.mult)
                            hb = pscr.tile([128, hidden], BF16, tag="hb")
                            nc.scalar.activation(hb[:], ps[:], mybir.ActivationFunctionType.Relu)
                            scr = pscr.tile([128, hidden], BF16, tag="scr")
                            a_f = pscr.tile([128, 1], F32, tag="af")
                            nc.vector.scalar_tensor_tensor(
                                out=scr[:], in0=hb[:], scalar=1.0, in1=gw_sb[:, 0, :],
                                op0=mybir.AluOpType.mult, op1=mybir.AluOpType.mult,
                                accum_out=a_f[:])
                            nc.vector.tensor_copy(a_arr[:, w, 0:1], a_f[:])
                        else:
                            hb = extg[:, c, 0:hh].bitcast(BF16)
                            nc.scalar.activation(hb, ps[:], mybir.ActivationFunctionType.Relu)
                            scr = pscr.tile([128, hidden], BF16, tag="scr")
                            nc.vector.scalar_tensor_tensor(
                                out=scr[:], in0=hb, scalar=1.0, in1=gw_sb[:, 1, :],
                                op0=mybir.AluOpType.mult, op1=mybir.AluOpType.mult,
                                accum_out=extg[:, c, B_SLOT:B_SLOT + 1])
                    if not own:
                        base = gi * PREP_GRP * 128
                        nc.sync.dma_start(
                            bass.AP(ext0, base * EXT_SLOTS,
                                    [[EXT_SLOTS, 128], [128 * EXT_SLOTS, gcnt], [1, B_SLOT + 1]]),
                            extg[:, 0:gcnt, :])

                for gi in range(ngrp + (1 if grp_rem else 0)):
                    prep_group(gi, PREP_GRP if gi < ngrp else grp_rem, xtg, xbg, False)
                for gi in range(ogrp + (1 if ogrp_rem else 0)):
                    prep_group(gi, PREP_GRP if gi < ogrp else ogrp_rem, xtog, xbog, True)

            # ---------------- edge phase (per layer) ----------------
            def emit_layer(l, table):
                lo_ap = table.ap()
                hi_ap = table.ap()[lo_split:, :]
                with tc.tile_pool(name=f"g{l}", bufs=2) as gp, \
                     tc.tile_pool(name=f"s{l}", bufs=3) as sp, \
                     tc.tile_pool(name=f"scr{l}", bufs=6) as scrp, \
                     tc.tile_pool(name=f"oh{l}", bufs=6) as ohp, \
                     tc.tile_pool(name=f"fin{l}", bufs=2) as fp, \
                     tc.tile_pool(name=f"psA{l}", bufs=2, space="PSUM") as psA, \
                     tc.tile_pool(name=f"psW{l}", bufs=2, space="PSUM") as psW:
                    # A_all: (a + gate_b) replicated across partitions, per window
                    for w in range(nwin):
                        awt_ps = psA.tile([1, 128], F32, tag="awt")
                        nc.tensor.matmul(awt_ps[:], a_arr[:, w, l:l + 1], identb_sb[:],
                                         start=True, stop=True)
                        awt_sb = sp.tile([1, 128], BF16, tag="awt_sb")
                        nc.vector.tensor_scalar(out=awt_sb[:], in0=awt_ps[:],
                                                scalar1=gb_sb[0:1, l:l + 1], scalar2=None,
                                                op0=mybir.AluOpType.add)
                        A_ps = psA.tile([128, 128], F32, tag="A")
                        nc.tensor.matmul(A_ps[:], onesb_sb[:], awt_sb[:],
                                         start=True, stop=True)
                        nc.vector.tensor_copy(A_all[:, w, :], A_ps[:])
                    def finalize(w):
                        rows = 128 if w < nwin - 1 else last_win_rows
                        if l == 0:
                            ext1 = fp.tile([128, B_SLOT + 1], F32, tag="ext1")
                            h1b = ext1[:, 0:hh].bitcast(BF16)
                            nc.vector.tensor_copy(h1b, acc[:, w, :])
                            scr = scrp.tile([128, hidden], BF16, tag="escr")
                            nc.vector.scalar_tensor_tensor(
                                out=scr[:], in0=h1b, scalar=1.0, in1=gw_sb[:, 3, :],
                                op0=mybir.AluOpType.mult, op1=mybir.AluOpType.mult,
                                accum_out=ext1[:, B_SLOT:B_SLOT + 1])
                            scr2 = scrp.tile([128, hidden], BF16, tag="escr2")
                            a_f = scrp.tile([128, 1], F32, tag="af1")
                            nc.vector.scalar_tensor_tensor(
                                out=scr2[:], in0=h1b, scalar=1.0, in1=gw_sb[:, 2, :],
                                op0=mybir.AluOpType.mult, op1=mybir.AluOpType.mult,
                                accum_out=a_f[:])
                            nc.vector.tensor_copy(a_arr[:, w, 1:2], a_f[:])
                            nc.sync.dma_start(agi.ap()[w * 128:w * 128 + rows, 0:B_SLOT + 1],
                                              ext1[0:rows, :])

                    # windows with no lo-burst: seed acc with rawsc; fully
                    # edgeless windows also finalize immediately
                    for w in range(nwin):
                        if (0, w) not in burst:
                            nc.vector.tensor_copy(acc[:, w, :], rawsc[:, w, :])
                            if (1, w) not in burst:
                                finalize(w)

                    qi = 0
                    W_ps = None
                    for (h, ts, nt) in [c for c in calls]:
                        ct = nt // 128
                        t0 = ts // 128
                        G = gp.tile([128, CT_MAX, EXT_SLOTS], F32, tag="G")
                        nc.gpsimd.dma_gather(
                            out_ap=G[:, 0:ct, :],
                            in_ap=(hi_ap if h else lo_ap),
                            idxs_ap=idxt[:, ts // 16:(ts + nt) // 16],
                            num_idxs=nt, num_idxs_reg=nt, elem_size=EXT_SLOTS,
                            single_packet=False, queue_num=qi % 4)
                        qi += 1
                        atok = sp.tile([128, CT_MAX], F32, tag="atok")
                        # split call into window-pure runs
                        runs = []
                        c = 0
                        while c < ct:
                            w = tiles_w[t0 + c]
                            c1 = c
                            while c1 < ct and tiles_w[t0 + c1] == w:
                                c1 += 1
                            runs.append((w, c, c1))
                            c = c1
                        for (w, c0, c1) in runs:
                            for c in range(c0, c1):
                                scr = scrp.tile([128, 128], BF16, tag="tscr")
                                nc.vector.scalar_tensor_tensor(
                                    out=scr[:], in0=iota_sb[:],
                                    scalar=rr_sb[:, t0 + c:t0 + c + 1],
                                    in1=A_all[:, w, :],
                                    op0=mybir.AluOpType.is_equal,
                                    op1=mybir.AluOpType.mult,
                                    accum_out=atok[:, c:c + 1])
                            n = c1 - c0
                            arg = sp.tile([128, CT_MAX], F32, tag="arg")
                            nc.vector.tensor_tensor(out=arg[:, 0:n], in0=atok[:, c0:c1],
                                                    in1=G[:, c0:c1, B_SLOT],
                                                    op=mybir.AluOpType.add)
                            gt = sp.tile([128, CT_MAX], F32, tag="gt")
                            nc.scalar.activation(gt[:, 0:n], arg[:, 0:n],
                                                 mybir.ActivationFunctionType.Tanh)
                            wt = sp.tile([128, CT_MAX], F32, tag="wt")
                            nc.vector.tensor_tensor(out=wt[:, 0:n], in0=gt[:, 0:n],
                                                    in1=es_sb[:, t0 + c0:t0 + c1],
                                                    op=mybir.AluOpType.mult)
                            bf, bl = burst[(h, w)]
                            if t0 + c0 == bf:
                                W_ps = psW.tile([128, hidden], F32, tag="W")
                            for c in range(c0, c1):
                                oh = ohp.tile([128, 128], BF16, tag="oh")
                                nc.vector.tensor_scalar(
                                    out=oh[:], in0=iota_sb[:],
                                    scalar1=rr_sb[:, t0 + c:t0 + c + 1],
                                    scalar2=wt[:, c - c0:c - c0 + 1],
                                    op0=mybir.AluOpType.is_equal,
                                    op1=mybir.AluOpType.mult)
                                nc.tensor.matmul(W_ps[:], oh[:], G[:, c, 0:hh].bitcast(BF16),
                                                 start=(t0 + c == bf),
                                                 stop=(t0 + c == bl))
                            if t0 + c1 - 1 == bl:
                                if h == 0:
                                    nc.vector.tensor_tensor(out=acc[:, w, :], in0=W_ps[:],
                                                            in1=rawsc[:, w, :],
                                                            op=mybir.AluOpType.add)
                                    if last_stream[w] == 0:
                                        finalize(w)
                                else:
                                    nc.vector.tensor_tensor(out=acc[:, w, :], in0=W_ps[:],
                                                            in1=acc[:, w, :],
                                                            op=mybir.AluOpType.add)
                                    finalize(w)

            phase = os.environ.get("KPHASE", "head")
            plvl = {"prep": 0, "l0": 1, "cc": 2, "l1": 3, "head": 4}[phase]
            if plvl >= 1:
                emit_layer(0, ext0)
            if plvl >= 2:
                nc.gpsimd.collective_compute(
                    "AllGather", mybir.AluOpType.bypass,
                    replica_groups=[list(range(ncores))],
                    ins=[agi.ap().opt()], outs=[ago.ap().opt()])
            if plvl >= 3:
                emit_layer(1, ago)
            if plvl < 4:
                with tc.tile_pool(name="zout", bufs=1) as zp:
                    o_z = zp.tile([128, out_ch], F32, tag="oz")
                    nc.vector.memset(o_z[:], 0.0)
                    for w in range(nwin):
                        rows = 128 if w < nwin - 1 else last_win_rows
                        nc.sync.dma_start(out.ap()[w * 128:w * 128 + rows, :],
                                          o_z[0:rows, :])
                return nc

            # ---------------- head: out = log_softmax(h @ t2^T + b) ----------
            with tc.tile_pool(name="head", bufs=3) as hp, \
                 tc.tile_pool(name="head_ps", bufs=4, space="PSUM") as hps:
                for w in range(nwin):
                    rows = 128 if w < nwin - 1 else last_win_rows
                    ht_ps = hps.tile([128, 128], F32, tag="ht")
                    nc.tensor.matmul(ht_ps[:], acc[:, w, :], ident_sb[:],
                                     start=True, stop=True)
                    ht_sb = hp.tile([128, 128], F32, tag="ht_sb")
                    nc.vector.tensor_copy(ht_sb[:], ht_ps[:])
                    o_ps = hps.tile([128, out_ch], F32, tag="ops")
                    nc.tensor.matmul(o_ps[:], ht_sb[:], t2wt_sb[:], start=True, stop=False)
                    nc.tensor.matmul(o_ps[:], ones_sb[:], t2b_sb[:], start=False, stop=True)
                    nm = hp.tile([128, 1], F32, tag="nm")
                    nc.vector.reduce_max(out=nm[:], in_=o_ps[:],
                                         axis=mybir.AxisListType.X, negate=True)
                    e_sb = hp.tile([128, out_ch], F32, tag="e")
                    nc.scalar.activation(e_sb[:], o_ps[:],
                                         mybir.ActivationFunctionType.Exp, bias=nm[:])
                    s_sb = hp.tile([128, 1], F32, tag="s")
                    nc.vector.reduce_sum(out=s_sb[:], in_=e_sb[:],
                                         axis=mybir.AxisListType.X)
                    ls = hp.tile([128, 1], F32, tag="ls")
                    nc.scalar.activation(ls[:], s_sb[:], mybir.ActivationFunctionType.Ln)
                    o_sb = hp.tile([128, out_ch], F32, tag="o")
                    nc.vector.tensor_scalar(out=o_sb[:], in0=o_ps[:],
                                            scalar1=nm[:], scalar2=ls[:],
                                            op0=mybir.AluOpType.add,
                                            op1=mybir.AluOpType.subtract)
                    nc.sync.dma_start(out.ap()[w * 128:w * 128 + rows, :], o_sb[0:rows, :])

    return nc


# ======================================================================
# Host driver
# ======================================================================

def _bf16(a):
    import ml_dtypes
    return np.asarray(a, dtype=ml_dtypes.bfloat16)


def _group_x(xT_pad, nrow_units, kt):
    # xT_pad: [in_ch+1, units*128] f32 -> xtg [ngrp, 128, PREP_GRP*kt, 128],
    # xbg [ngrp, 1, PREP_GRP, 128] (ones row)
    in_ch = (xT_pad.shape[0] - 1)
    ngrp_t = (nrow_units + PREP_GRP - 1) // PREP_GRP
    pad_units = ngrp_t * PREP_GRP
    xp = np.zeros((in_ch + 1, pad_units * 128), np.float32)
    xp[:, :xT_pad.shape[1]] = xT_pad
    # [in, u, 128] -> [u, in, 128]
    xr = xp[:in_ch].reshape(in_ch, pad_units, 128).transpose(1, 0, 2)
    # [g, c, k, p, r] with in = k*128+p
    xg = xr.reshape(ngrp_t, PREP_GRP, kt, 128, 128)
    xtg = np.ascontiguousarray(xg.transpose(0, 3, 1, 2, 4)).reshape(
        ngrp_t, 128, PREP_GRP * kt, 128)
    xb = xp[in_ch].reshape(ngrp_t, 1, PREP_GRP, 128)
    return _bf16(xtg), _bf16(np.ascontiguousarray(xb))


def kernel_run(x, edge_index, t1_w, t1_b, gate_w, gate_b, t2_w, t2_b,
               n_nodes=N_NODES, in_ch=IN_CH, hidden=HIDDEN, out_ch=OUT_CH,
               eps=EPS, ncores=NCORES, lo_split=None, trace=False):
    _install_profile_hook()
    from concourse import bass_utils

    if lo_split is None:
        lo_split = min(25000, ((n_nodes + 1) // 2 + 127) // 128 * 128)
    meta = preprocess(edge_index, n_nodes, ncores, lo_split)
    nwin = meta["nwin"]
    r_per = n_nodes // ncores
    nchunk_tot = (n_nodes + 127) // 128
    kt = in_ch // 128

    nc = build_kernel(meta, n_nodes, in_ch, hidden, out_ch, eps, lo_split, ncores)
    nc.finalize()

    # host arrays
    x = np.asarray(x, np.float32)
    xT = np.concatenate([x.T, np.ones((1, x.shape[0]), np.float32)], axis=0)  # [in+1, N]
    pad_n = nchunk_tot * 128
    xT_pad = np.zeros((in_ch + 1, pad_n), np.float32)
    xT_pad[:, :n_nodes] = xT
    xtg_h, xbg_h = _group_x(xT_pad, nchunk_tot, kt)

    t1wt_h = _bf16(np.concatenate([np.asarray(t1_w, np.float32).T,
                                   np.asarray(t1_b, np.float32)[None, :]], axis=0))
    gw = np.asarray(gate_w, np.float32)
    gwrep_h = _bf16(np.stack([
        np.tile(gw[0, :hidden][None, :], (128, 1)),
        np.tile(gw[0, hidden:][None, :], (128, 1)),
        np.tile(gw[1, :hidden][None, :], (128, 1)),
        np.tile(gw[1, hidden:][None, :], (128, 1))]))
    gbrep_h = np.tile(np.asarray(gate_b, np.float32)[None, :], (128, 1))
    t2wt_h = np.ascontiguousarray(np.asarray(t2_w, np.float32).T)
    t2b_h = np.asarray(t2_b, np.float32)[None, :]
    iota_h = _bf16(np.tile(np.arange(128, dtype=np.float32)[None, :], (128, 1)))
    identb_h = _bf16(np.eye(128, dtype=np.float32))
    ident_h = np.eye(128, dtype=np.float32)
    onesb_h = _bf16(np.ones((1, 128), np.float32))
    ones_h = np.ones((1, 128), np.float32)

    in_maps = []
    for c in range(ncores):
        sl = np.zeros((in_ch + 1, nwin * 128), np.float32)
        take = min(nwin * 128, xT.shape[1] - c * r_per)
        sl[:, :take] = xT[:, c * r_per: c * r_per + take]
        xtog_h, xbog_h = _group_x(sl, nwin, kt)
        in_maps.append({
            "xtg": xtg_h, "xbg": xbg_h, "xtog": xtog_h, "xbog": xbog_h,
            "t1wt": t1wt_h, "gwrep": gwrep_h, "gbrep": gbrep_h,
            "t2wt": t2wt_h, "t2b": t2b_h,
            "iota": iota_h, "identb": identb_h, "ident": ident_h,
            "onesb": onesb_h, "ones": ones_h,
            "idx": meta["idx_dev"][c], "rr": meta["rr_dev"][c],
            "es": meta["es_dev"][c],
        })

    res = bass_utils.run_bass_kernel_spmd(
        nc, in_maps, core_ids=list(range(ncores)), trace=trace)
    outp = np.concatenate([res.results[c]["out"] for c in range(ncores)], axis=0)
    return outp[:n_nodes], res


def kernel(**inputs):
    x = inputs["x"]
    edge_index = inputs["edge_index"]
    outp, _ = kernel_run(
        x, edge_index, inputs["t1_w"], inputs["t1_b"], inputs["gate_w"],
        inputs["gate_b"], inputs["t2_w"], inputs["t2_b"])
    return np.asarray(outp, np.float32)


# revision 21
# speedup vs baseline: 1.3395x; 1.1206x over previous
"""FAGCN forward on 8 TRN2 NeuronCores (Bass/Tile) — v2.

Sharding: row-partition of nodes, 8 ways. The dense input projection
(h = relu(x @ t1^T + b)) is replicated on every core into a 512B-stride
gather table [h bf16 x128 | b f32 | pad]. Per layer the edge phase is a
two-stream token walk ([all-lo windows][all-hi windows], int16 gather
indices split at lo_split): big dma_gather calls (6144 edges) fetch
source rows; per 128-edge tile a bf16 one-hot (4x DVE mode) both
recovers a[row] (tensor_tensor_reduce against a partition-replicated
a-broadcast) and scatter-adds w*h[col] into a per-window PSUM
accumulator via TensorE. Window results accumulate in SBUF across the
two streams. Between layers the owned rows are AllGathered. The head
(t2 matmul + log_softmax) runs as a final pass so the activation table
is not thrashed.
"""

import os
import sys
import numpy as np

sys.path.insert(0, "/opt/trn_rl_repo")

import concourse.bass as bass
import concourse.bacc as bacc
import concourse.mybir as mybir
import concourse.tile as tile
from concourse import library_config

F32 = mybir.dt.float32
BF16 = mybir.dt.bfloat16
I16 = mybir.dt.int16

# problem constants (self-contained per contract)
N_NODES = 50000
IN_CH = 256
HIDDEN = 128
OUT_CH = 64
EPS = 0.3
NCORES = 8
CALL_TOKENS = int(os.environ.get("KCT", "2048"))
CT_MAX = CALL_TOKENS // 128
EXT_SLOTS = 128   # 512B gather record
B_SLOT = 64       # f32 slot holding the gate b-term
PREP_GRP = 8


def _install_profile_hook():
    import types
    name = "antenv.axon_hooks"
    if name in sys.modules:
        return
    try:
        import trn_agent_boot.trn_boot as tb
        hook = tb._ntff_profile_via_ctypes("/opt/axon/libaxon_pjrt.so")
    except Exception:
        hook = None
    mod = types.ModuleType(name)
    mod._hook = hook
    mod.get_axon_ntff_profile_hook = lambda: mod._hook
    mod.set_axon_ntff_profile_hook = lambda h: setattr(mod, "_hook", h)
    sys.modules[name] = mod


# ======================================================================
# Host preprocessing: SPMD token streams + per-core data
# ======================================================================

def preprocess(edge_index, n_nodes, ncores, lo_split):
    row = np.asarray(edge_index[0], dtype=np.int64)
    col = np.asarray(edge_index[1], dtype=np.int64)
    E = row.shape[0]
    r_per = n_nodes // ncores
    nwin = (r_per + 127) // 128

    deg = np.bincount(row, minlength=n_nodes).astype(np.float64)
    dinv = np.where(deg > 0, 1.0 / np.sqrt(np.maximum(deg, 1.0)), 0.0)
    escale_all = (dinv[row] * dinv[col]).astype(np.float32)

    core = row // r_per
    lrow = row - core * r_per
    win = lrow // 128
    is_hi = (col >= lo_split).astype(np.int64)

    # stream order: core, then stream (lo/hi), then window, then lrow
    order = np.lexsort((lrow, win, is_hi, core))
    core_s, win_s, hi_s = core[order], win[order], is_hi[order]
    lrow_s, col_s, esc_s = lrow[order], col[order], escale_all[order]

    key = (core_s * 2 + hi_s) * nwin + win_s
    cnt = np.bincount(key, minlength=ncores * 2 * nwin).reshape(ncores, 2, nwin)
    sec_len = ((cnt.max(axis=0) + 127) // 128) * 128  # [2, nwin]
    L_lo = int(sec_len[0].sum())
    L_hi = int(sec_len[1].sum())
    e_tok = L_lo + L_hi
    sec_start = np.zeros((2, nwin), np.int64)
    sec_start[0] = np.concatenate([[0], np.cumsum(sec_len[0])[:-1]])
    sec_start[1] = L_lo + np.concatenate([[0], np.cumsum(sec_len[1])[:-1]])

    col16 = np.zeros((ncores, e_tok), np.int16)
    rowrel = np.zeros((ncores, e_tok), np.float32)
    esc = np.zeros((ncores, e_tok), np.float32)

    grp_first = np.zeros(ncores * 2 * nwin + 1, np.int64)
    np.cumsum(cnt.reshape(-1), out=grp_first[1:])
    rank = np.arange(E) - grp_first[key]
    dest = sec_start[hi_s, win_s] + rank
    cval = np.where(hi_s == 1, col_s - lo_split, col_s).astype(np.int16)
    col16[core_s, dest] = cval
    rowrel[core_s, dest] = (lrow_s - win_s * 128).astype(np.float32)
    esc[core_s, dest] = esc_s

    # gather calls per stream
    calls = []  # (stream, ts, nt)
    for h, base, L in ((0, 0, L_lo), (1, L_lo, L_hi)):
        off = 0
        while off < L:
            nt = min(CALL_TOKENS, L - off)
            calls.append((h, base + off, nt))
            off += nt

    idx_dev = np.zeros((ncores, 128, e_tok // 16), np.int16)
    for (h, ts, nt) in calls:
        blk = col16[:, ts:ts + nt].reshape(ncores, nt // 16, 16)
        blk = np.ascontiguousarray(np.transpose(blk, (0, 2, 1)))
        idx_dev[:, :, ts // 16:(ts + nt) // 16] = np.tile(blk, (1, 8, 1))
    rr_dev = np.ascontiguousarray(rowrel.reshape(ncores, -1, 128).transpose(0, 2, 1))
    es_dev = np.ascontiguousarray(esc.reshape(ncores, -1, 128).transpose(0, 2, 1))

    # transposed one-hot per tile: eht[t, r, k] = 1 iff rowrel[t*128+k] == r
    import ml_dtypes
    ntile = e_tok // 128
    eht = np.zeros((ncores, ntile * 128 * 128), ml_dtypes.bfloat16)
    ti = np.arange(e_tok) // 128
    ki = np.arange(e_tok) % 128
    for c in range(ncores):
        flat = ti * 16384 + rowrel[c].astype(np.int64) * 128 + ki
        eht[c][flat] = 1.0
    eht = eht.reshape(ncores, ntile, 128, 128)

    return {
        "nwin": nwin, "e_tok": e_tok, "sec_len": sec_len, "calls": calls,
        "idx_dev": idx_dev, "rr_dev": rr_dev, "es_dev": es_dev, "eht_dev": eht,
    }


# ======================================================================
# Kernel builder
# ======================================================================

def build_kernel(meta, n_nodes, in_ch, hidden, out_ch, eps, lo_split, ncores):
    nwin = meta["nwin"]
    e_tok = meta["e_tok"]
    sec_len = meta["sec_len"]
    calls = meta["calls"]
    nchunk_tot = (n_nodes + 127) // 128
    r_per = n_nodes // ncores
    last_win_rows = r_per - 128 * (nwin - 1)
    kt = in_ch // 128
    hh = hidden // 2  # f32 slots holding the bf16 h vector

    # tile -> window map, and burst boundaries per (stream, window)
    tiles_w = []
    burst = {}  # (h, w) -> (gfirst, glast) in global tile idx
    for h in range(2):
        for w in range(nwin):
            ntl = int(sec_len[h, w]) // 128
            if ntl == 0:
                continue
            g0 = len(tiles_w)
            tiles_w.extend([w] * ntl)
            burst[(h, w)] = (g0, g0 + ntl - 1)
    assert len(tiles_w) == e_tok // 128
    last_stream = {}
    for w in range(nwin):
        last_stream[w] = 1 if (1, w) in burst else 0

    ngrp = nchunk_tot // PREP_GRP
    grp_rem = nchunk_tot - ngrp * PREP_GRP
    ogrp = nwin // PREP_GRP
    ogrp_rem = nwin - ogrp * PREP_GRP

    nc = bacc.Bacc("TRN2", target_bir_lowering=False, debug=False,
                   num_devices=ncores, num_swdge_queues=4)

    # ---- I/O ----
    # xtg: host-prearranged [group, 128p, grp*kt, 128] bf16 (+ ones row separately)
    xtg = nc.dram_tensor("xtg", [ngrp + (1 if grp_rem else 0), 128, PREP_GRP * kt, 128], BF16, kind="ExternalInput")
    xbg = nc.dram_tensor("xbg", [ngrp + (1 if grp_rem else 0), 1, PREP_GRP, 128], BF16, kind="ExternalInput")
    xtog = nc.dram_tensor("xtog", [ogrp + (1 if ogrp_rem else 0), 128, PREP_GRP * kt, 128], BF16, kind="ExternalInput")
    xbog = nc.dram_tensor("xbog", [ogrp + (1 if ogrp_rem else 0), 1, PREP_GRP, 128], BF16, kind="ExternalInput")
    t1wt = nc.dram_tensor("t1wt", [in_ch + 1, hidden], BF16, kind="ExternalInput")
    gwrep = nc.dram_tensor("gwrep", [4, 128, hidden], BF16, kind="ExternalInput")
    gbrep = nc.dram_tensor("gbrep", [128, 2], F32, kind="ExternalInput")
    t2wt = nc.dram_tensor("t2wt", [hidden, out_ch], F32, kind="ExternalInput")
    t2b = nc.dram_tensor("t2b", [1, out_ch], F32, kind="ExternalInput")
    iota_in = nc.dram_tensor("iota", [128, 128], BF16, kind="ExternalInput")
    ident_in = nc.dram_tensor("ident", [128, 128], F32, kind="ExternalInput")
    ones_in = nc.dram_tensor("ones", [1, 128], F32, kind="ExternalInput")
    idx_in = nc.dram_tensor("idx", [128, e_tok // 16], I16, kind="ExternalInput")
    eht_in = nc.dram_tensor("eht", [e_tok // 128, 128, 128], BF16, kind="ExternalInput")
    rr_in = nc.dram_tensor("rr", [128, e_tok // 128], F32, kind="ExternalInput")
    es_in = nc.dram_tensor("es", [128, e_tok // 128], F32, kind="ExternalInput")
    out = nc.dram_tensor("out", [r_per, out_ch], F32, kind="ExternalOutput")

    ext0 = nc.dram_tensor("ext0", [nchunk_tot * 128, EXT_SLOTS], F32)
    agi = nc.dram_tensor("agi", [r_per, EXT_SLOTS], F32)
    ago = nc.dram_tensor("ago", [r_per * ncores, EXT_SLOTS], F32)

    with tile.TileContext(nc) as tc:
        nc.gpsimd.load_library(library_config.mlp)
        with tc.tile_pool(name="consts", bufs=1) as cp:
            t1wt_sb = cp.tile([128, kt, hidden], BF16, tag="t1wt")
            nc.sync.dma_start(t1wt_sb[:], bass.AP(t1wt, 0, [[hidden, 128], [128 * hidden, kt], [1, hidden]]))
            t1b_sb = cp.tile([1, hidden], BF16, tag="t1b")
            nc.sync.dma_start(t1b_sb[:], t1wt.ap()[in_ch:in_ch + 1, :])
            gw_sb = cp.tile([128, 4, hidden], BF16, tag="gw")
            nc.sync.dma_start(gw_sb[:], bass.AP(gwrep, 0, [[hidden, 128], [128 * hidden, 4], [1, hidden]]))
            gb_sb = cp.tile([128, 2], F32, tag="gb")
            nc.sync.dma_start(gb_sb[:], gbrep.ap())
            t2wt_sb = cp.tile([128, out_ch], F32, tag="t2wt")
            nc.sync.dma_start(t2wt_sb[:], t2wt.ap())
            t2b_sb = cp.tile([1, out_ch], F32, tag="t2b")
            nc.sync.dma_start(t2b_sb[:], t2b.ap())
            iota_sb = cp.tile([128, 128], BF16, tag="iota")
            nc.sync.dma_start(iota_sb[:], iota_in.ap())
            ident_sb = cp.tile([128, 128], F32, tag="ident")
            nc.sync.dma_start(ident_sb[:], ident_in.ap())
            ones_sb = cp.tile([1, 128], F32, tag="ones")
            nc.sync.dma_start(ones_sb[:], ones_in.ap())
            idxt = cp.tile([128, e_tok // 16], I16, tag="idxt")
            nc.sync.dma_start(idxt[:], idx_in.ap())
            rr_sb = cp.tile([128, e_tok // 128], F32, tag="rr")
            nc.sync.dma_start(rr_sb[:], rr_in.ap())
            es_sb = cp.tile([128, e_tok // 128], F32, tag="es")
            nc.sync.dma_start(es_sb[:], es_in.ap())

            rawsc = cp.tile([128, nwin, hidden], F32, tag="rawsc")
            acc = cp.tile([128, nwin, hidden], F32, tag="acc")
            a_arr = cp.tile([128, nwin, 2], BF16, tag="a_arr")

            # ---------------- prep: replicated gather table ----------------
            with tc.tile_pool(name="prep", bufs=3) as pp, \
                 tc.tile_pool(name="prep_s", bufs=6) as pscr, \
                 tc.tile_pool(name="prep_ps", bufs=4, space="PSUM") as pps:

                def prep_group(gi, gcnt, xt_t, xb_t, own):
                    xt_sb = pp.tile([128, PREP_GRP * kt, 128], BF16, tag="xt")
                    nc.sync.dma_start(xt_sb[:, 0:gcnt * kt, :], xt_t.ap()[gi, :, 0:gcnt * kt, :])
                    xb_sb = pp.tile([1, PREP_GRP, 128], BF16, tag="xb")
                    nc.sync.dma_start(xb_sb[:, 0:gcnt, :], xb_t.ap()[gi, :, 0:gcnt, :])
                    if not own:
                        extg = pp.tile([128, PREP_GRP, B_SLOT + 1], F32, tag="extg")
                    for c in range(gcnt):
                        ps = pps.tile([128, hidden], F32, tag="h0ps")
                        for k in range(kt):
                            nc.tensor.matmul(ps[:], xt_sb[:, c * kt + k, :], t1wt_sb[:, k, :],
                                             start=(k == 0), stop=False)
                        nc.tensor.matmul(ps[:], xb_sb[:, c, :], t1b_sb[:], start=False, stop=True)
                        if own:
                            w = gi * PREP_GRP + c
                            nc.vector.tensor_scalar(out=rawsc[:, w, :], in0=ps[:],
                                                    scalar1=0.0, scalar2=eps,
                                                    op0=mybir.AluOpType.max,
                                                    op1=mybir.AluOpType.mult)
                            hb = pscr.tile([128, hidden], BF16, tag="hb")
                            nc.scalar.activation(hb[:], ps[:], mybir.ActivationFunctionType.Relu)
                            scr = pscr.tile([128, hidden], BF16, tag="scr")
                            a_f = pscr.tile([128, 1], F32, tag="af")
                            nc.vector.scalar_tensor_tensor(
                                out=scr[:], in0=hb[:], scalar=1.0, in1=gw_sb[:, 0, :],
                                op0=mybir.AluOpType.mult, op1=mybir.AluOpType.mult,
                                accum_out=a_f[:])
                            nc.vector.tensor_scalar(out=a_arr[:, w, 0:1], in0=a_f[:],
                                                    scalar1=gb_sb[:, 0:1], scalar2=None,
                                                    op0=mybir.AluOpType.add)
                        else:
                            hb = extg[:, c, 0:hh].bitcast(BF16)
                            nc.scalar.activation(hb, ps[:], mybir.ActivationFunctionType.Relu)
                            scr = pscr.tile([128, hidden], BF16, tag="scr")
                            nc.vector.scalar_tensor_tensor(
                                out=scr[:], in0=hb, scalar=1.0, in1=gw_sb[:, 1, :],
                                op0=mybir.AluOpType.mult, op1=mybir.AluOpType.mult,
                                accum_out=extg[:, c, B_SLOT:B_SLOT + 1])
                    if not own:
                        base = gi * PREP_GRP * 128
                        nc.sync.dma_start(
                            bass.AP(ext0, base * EXT_SLOTS,
                                    [[EXT_SLOTS, 128], [128 * EXT_SLOTS, gcnt], [1, B_SLOT + 1]]),
                            extg[:, 0:gcnt, :])

                for gi in range(ngrp + (1 if grp_rem else 0)):
                    prep_group(gi, PREP_GRP if gi < ngrp else grp_rem, xtg, xbg, False)
                for gi in range(ogrp + (1 if ogrp_rem else 0)):
                    prep_group(gi, PREP_GRP if gi < ogrp else ogrp_rem, xtog, xbog, True)

            # ---------------- edge phase (per layer) ----------------
            def emit_layer(l, table):
                lo_ap = table.ap()
                hi_ap = table.ap()[lo_split:, :]
                with tc.tile_pool(name=f"g{l}", bufs=int(os.environ.get("KGB", "2"))) as gp, \
                     tc.tile_pool(name=f"e{l}", bufs=int(os.environ.get("KEB", "2"))) as ep, \
                     tc.tile_pool(name=f"s{l}", bufs=3) as sp, \
                     tc.tile_pool(name=f"scr{l}", bufs=6) as scrp, \
                     tc.tile_pool(name=f"oh{l}", bufs=6) as ohp, \
                     tc.tile_pool(name=f"fin{l}", bufs=2) as fp, \
                     tc.tile_pool(name=f"psT{l}", bufs=3, space="PSUM") as psT, \
                     tc.tile_pool(name=f"psW{l}", bufs=2, space="PSUM") as psW:
                    def finalize(w):
                        rows = 128 if w < nwin - 1 else last_win_rows
                        if l == 0:
                            ext1 = fp.tile([128, B_SLOT + 1], F32, tag="ext1")
                            h1b = ext1[:, 0:hh].bitcast(BF16)
                            nc.vector.tensor_copy(h1b, acc[:, w, :])
                            scr = scrp.tile([128, hidden], BF16, tag="escr")
                            nc.vector.scalar_tensor_tensor(
                                out=scr[:], in0=h1b, scalar=1.0, in1=gw_sb[:, 3, :],
                                op0=mybir.AluOpType.mult, op1=mybir.AluOpType.mult,
                                accum_out=ext1[:, B_SLOT:B_SLOT + 1])
                            scr2 = scrp.tile([128, hidden], BF16, tag="escr2")
                            a_f = scrp.tile([128, 1], F32, tag="af1")
                            nc.vector.scalar_tensor_tensor(
                                out=scr2[:], in0=h1b, scalar=1.0, in1=gw_sb[:, 2, :],
                                op0=mybir.AluOpType.mult, op1=mybir.AluOpType.mult,
                                accum_out=a_f[:])
                            nc.vector.tensor_scalar(out=a_arr[:, w, 1:2], in0=a_f[:],
                                                    scalar1=gb_sb[:, 1:2], scalar2=None,
                                                    op0=mybir.AluOpType.add)
                            nc.sync.dma_start(agi.ap()[w * 128:w * 128 + rows, 0:B_SLOT + 1],
                                              ext1[0:rows, :])

                    # windows with no lo-burst: seed acc with rawsc; fully
                    # edgeless windows also finalize immediately
                    for w in range(nwin):
                        if (0, w) not in burst:
                            nc.vector.tensor_copy(acc[:, w, :], rawsc[:, w, :])
                            if (1, w) not in burst:
                                finalize(w)

                    qi = 0
                    W_ps = None
                    for (h, ts, nt) in [c for c in calls]:
                        ct = nt // 128
                        t0 = ts // 128
                        G = gp.tile([128, CT_MAX, EXT_SLOTS], F32, tag="G")
                        nc.gpsimd.dma_gather(
                            out_ap=G[:, 0:ct, :],
                            in_ap=(hi_ap if h else lo_ap),
                            idxs_ap=idxt[:, ts // 16:(ts + nt) // 16],
                            num_idxs=nt, num_idxs_reg=nt, elem_size=EXT_SLOTS,
                            single_packet=False, queue_num=qi % 4)
                        qi += 1
                        ehT = ep.tile([128, CT_MAX, 128], BF16, tag="ehT")
                        nc.sync.dma_start(
                            ehT[:, 0:ct, :],
                            bass.AP(eht_in, t0 * 16384,
                                    [[128, 128], [16384, ct], [1, 128]]))
                        atokP = psT.tile([128, CT_MAX], F32, tag="atokP")
                        # split call into window-pure runs
                        runs = []
                        c = 0
                        while c < ct:
                            w = tiles_w[t0 + c]
                            c1 = c
                            while c1 < ct and tiles_w[t0 + c1] == w:
                                c1 += 1
                            runs.append((w, c, c1))
                            c = c1
                        for (w, c0, c1) in runs:
                            for c in range(c0, c1):
                                nc.tensor.matmul(atokP[:, c:c + 1], ehT[:, c, :],
                                                 a_arr[:, w, l:l + 1],
                                                 start=True, stop=True)
                            n = c1 - c0
                            arg = sp.tile([128, CT_MAX], F32, tag="arg")
                            nc.vector.tensor_tensor(out=arg[:, 0:n], in0=atokP[:, c0:c1],
                                                    in1=G[:, c0:c1, B_SLOT],
                                                    op=mybir.AluOpType.add)
                            gt = sp.tile([128, CT_MAX], F32, tag="gt")
                            nc.scalar.activation(gt[:, 0:n], arg[:, 0:n],
                                                 mybir.ActivationFunctionType.Tanh)
                            wt = sp.tile([128, CT_MAX], F32, tag="wt")
                            nc.vector.tensor_tensor(out=wt[:, 0:n], in0=gt[:, 0:n],
                                                    in1=es_sb[:, t0 + c0:t0 + c1],
                                                    op=mybir.AluOpType.mult)
                            bf, bl = burst[(h, w)]
                            if t0 + c0 == bf:
                                W_ps = psW.tile([128, hidden], F32, tag="W")
                            for c in range(c0, c1):
                                oh = ohp.tile([128, 128], BF16, tag="oh")
                                nc.vector.tensor_scalar(
                                    out=oh[:], in0=iota_sb[:],
                                    scalar1=rr_sb[:, t0 + c:t0 + c + 1],
                                    scalar2=wt[:, c - c0:c - c0 + 1],
                                    op0=mybir.AluOpType.is_equal,
                                    op1=mybir.AluOpType.mult)
                                nc.tensor.matmul(W_ps[:], oh[:], G[:, c, 0:hh].bitcast(BF16),
                                                 start=(t0 + c == bf),
                                                 stop=(t0 + c == bl))
                            if t0 + c1 - 1 == bl:
                                if h == 0:
                                    nc.vector.tensor_tensor(out=acc[:, w, :], in0=W_ps[:],
                                                            in1=rawsc[:, w, :],
                                                            op=mybir.AluOpType.add)
                                    if last_stream[w] == 0:
                                        finalize(w)
                                else:
                                    nc.vector.tensor_tensor(out=acc[:, w, :], in0=W_ps[:],
                                                            in1=acc[:, w, :],
                                                            op=mybir.AluOpType.add)
                                    finalize(w)

            phase = os.environ.get("KPHASE", "head")
            plvl = {"prep": 0, "l0": 1, "cc": 2, "l1": 3, "head": 4}[phase]
            if plvl >= 1:
                emit_layer(0, ext0)
            if plvl >= 2:
                nc.gpsimd.collective_compute(
                    "AllGather", mybir.AluOpType.bypass,
                    replica_groups=[list(range(ncores))],
                    ins=[agi.ap().opt()], outs=[ago.ap().opt()])
            if plvl >= 3:
                emit_layer(1, ago)
            if plvl < 4:
                with tc.tile_pool(name="zout", bufs=1) as zp:
                    o_z = zp.tile([128, out_ch], F32, tag="oz")
                    nc.vector.memset(o_z[:], 0.0)
                    for w in range(nwin):
                        rows = 128 if w < nwin - 1 else last_win_rows
                        nc.sync.dma_start(out.ap()[w * 128:w * 128 + rows, :],
                                          o_z[0:rows, :])
                return nc

            # ---------------- head: out = log_softmax(h @ t2^T + b) ----------
            # two passes so the Act engine loads the Exp/Ln tables once each
            with tc.tile_pool(name="head", bufs=4) as hp, \
                 tc.tile_pool(name="head_ps", bufs=4, space="PSUM") as hps:
                o_all = cp.tile([128, nwin, out_ch], F32, tag="o_all")
                nm_all = cp.tile([128, nwin], F32, tag="nm_all")
                s_all = cp.tile([128, nwin], F32, tag="s_all")
                for w in range(nwin):
                    ht_ps = hps.tile([128, 128], F32, tag="ht")
                    nc.tensor.matmul(ht_ps[:], acc[:, w, :], ident_sb[:],
                                     start=True, stop=True)
                    ht_sb = hp.tile([128, 128], F32, tag="ht_sb")
                    nc.vector.tensor_copy(ht_sb[:], ht_ps[:])
                    o_ps = hps.tile([128, out_ch], F32, tag="ops")
                    nc.tensor.matmul(o_ps[:], ht_sb[:], t2wt_sb[:], start=True, stop=False)
                    nc.tensor.matmul(o_ps[:], ones_sb[:], t2b_sb[:], start=False, stop=True)
                    nc.vector.reduce_max(out=nm_all[:, w:w + 1], in_=o_ps[:],
                                         axis=mybir.AxisListType.X, negate=True)
                    e_sb = hp.tile([128, out_ch], F32, tag="e")
                    nc.scalar.activation(e_sb[:], o_ps[:],
                                         mybir.ActivationFunctionType.Exp,
                                         bias=nm_all[:, w:w + 1])
                    nc.vector.reduce_sum(out=s_all[:, w:w + 1], in_=e_sb[:],
                                         axis=mybir.AxisListType.X)
                    nc.vector.tensor_copy(o_all[:, w, :], o_ps[:])
                ls_all = cp.tile([128, nwin], F32, tag="ls_all")
                nc.scalar.activation(ls_all[:], s_all[:], mybir.ActivationFunctionType.Ln)
                for w in range(nwin):
                    rows = 128 if w < nwin - 1 else last_win_rows
                    o_sb = hp.tile([128, out_ch], F32, tag="o")
                    nc.vector.tensor_scalar(out=o_sb[:], in0=o_all[:, w, :],
                                            scalar1=nm_all[:, w:w + 1],
                                            scalar2=ls_all[:, w:w + 1],
                                            op0=mybir.AluOpType.add,
                                            op1=mybir.AluOpType.subtract)
                    nc.sync.dma_start(out.ap()[w * 128:w * 128 + rows, :], o_sb[0:rows, :])

    return nc


# ======================================================================
# Host driver
# ======================================================================

def _bf16(a):
    import ml_dtypes
    return np.asarray(a, dtype=ml_dtypes.bfloat16)


def _group_x(xT_pad, nrow_units, kt):
    # xT_pad: [in_ch+1, units*128] f32 -> xtg [ngrp, 128, PREP_GRP*kt, 128],
    # xbg [ngrp, 1, PREP_GRP, 128] (ones row)
    in_ch = (xT_pad.shape[0] - 1)
    ngrp_t = (nrow_units + PREP_GRP - 1) // PREP_GRP
    pad_units = ngrp_t * PREP_GRP
    xp = np.zeros((in_ch + 1, pad_units * 128), np.float32)
    xp[:, :xT_pad.shape[1]] = xT_pad
    # [in, u, 128] -> [u, in, 128]
    xr = xp[:in_ch].reshape(in_ch, pad_units, 128).transpose(1, 0, 2)
    # [g, c, k, p, r] with in = k*128+p
    xg = xr.reshape(ngrp_t, PREP_GRP, kt, 128, 128)
    xtg = np.ascontiguousarray(xg.transpose(0, 3, 1, 2, 4)).reshape(
        ngrp_t, 128, PREP_GRP * kt, 128)
    xb = xp[in_ch].reshape(ngrp_t, 1, PREP_GRP, 128)
    return _bf16(xtg), _bf16(np.ascontiguousarray(xb))


def kernel_run(x, edge_index, t1_w, t1_b, gate_w, gate_b, t2_w, t2_b,
               n_nodes=N_NODES, in_ch=IN_CH, hidden=HIDDEN, out_ch=OUT_CH,
               eps=EPS, ncores=NCORES, lo_split=None, trace=False):
    _install_profile_hook()
    from concourse import bass_utils

    if lo_split is None:
        lo_split = min(25000, ((n_nodes + 1) // 2 + 127) // 128 * 128)
    meta = preprocess(edge_index, n_nodes, ncores, lo_split)
    nwin = meta["nwin"]
    r_per = n_nodes // ncores
    nchunk_tot = (n_nodes + 127) // 128
    kt = in_ch // 128

    nc = build_kernel(meta, n_nodes, in_ch, hidden, out_ch, eps, lo_split, ncores)
    nc.finalize()

    # host arrays
    x = np.asarray(x, np.float32)
    xT = np.concatenate([x.T, np.ones((1, x.shape[0]), np.float32)], axis=0)  # [in+1, N]
    pad_n = nchunk_tot * 128
    xT_pad = np.zeros((in_ch + 1, pad_n), np.float32)
    xT_pad[:, :n_nodes] = xT
    xtg_h, xbg_h = _group_x(xT_pad, nchunk_tot, kt)

    t1wt_h = _bf16(np.concatenate([np.asarray(t1_w, np.float32).T,
                                   np.asarray(t1_b, np.float32)[None, :]], axis=0))
    gw = np.asarray(gate_w, np.float32)
    gwrep_h = _bf16(np.stack([
        np.tile(gw[0, :hidden][None, :], (128, 1)),
        np.tile(gw[0, hidden:][None, :], (128, 1)),
        np.tile(gw[1, :hidden][None, :], (128, 1)),
        np.tile(gw[1, hidden:][None, :], (128, 1))]))
    gbrep_h = np.tile(np.asarray(gate_b, np.float32)[None, :], (128, 1))
    t2wt_h = np.ascontiguousarray(np.asarray(t2_w, np.float32).T)
    t2b_h = np.asarray(t2_b, np.float32)[None, :]
    iota_h = _bf16(np.tile(np.arange(128, dtype=np.float32)[None, :], (128, 1)))
    ident_h = np.eye(128, dtype=np.float32)
    ones_h = np.ones((1, 128), np.float32)

    in_maps = []
    for c in range(ncores):
        sl = np.zeros((in_ch + 1, nwin * 128), np.float32)
        take = min(nwin * 128, xT.shape[1] - c * r_per)
        sl[:, :take] = xT[:, c * r_per: c * r_per + take]
        xtog_h, xbog_h = _group_x(sl, nwin, kt)
        in_maps.append({
            "xtg": xtg_h, "xbg": xbg_h, "xtog": xtog_h, "xbog": xbog_h,
            "t1wt": t1wt_h, "gwrep": gwrep_h, "gbrep": gbrep_h,
            "t2wt": t2wt_h, "t2b": t2b_h,
            "iota": iota_h, "ident": ident_h, "ones": ones_h,
            "idx": meta["idx_dev"][c], "rr": meta["rr_dev"][c],
            "es": meta["es_dev"][c], "eht": meta["eht_dev"][c],
        })

    res = bass_utils.run_bass_kernel_spmd(
        nc, in_maps, core_ids=list(range(ncores)), trace=trace)
    outp = np.concatenate([res.results[c]["out"] for c in range(ncores)], axis=0)
    return outp[:n_nodes], res


def kernel(**inputs):
    x = inputs["x"]
    edge_index = inputs["edge_index"]
    outp, _ = kernel_run(
        x, edge_index, inputs["t1_w"], inputs["t1_b"], inputs["gate_w"],
        inputs["gate_b"], inputs["t2_w"], inputs["t2_b"])
    return np.asarray(outp, np.float32)


# revision 23
# speedup vs baseline: 1.8597x; 1.3884x over previous
"""FAGCN forward on 8 TRN2 NeuronCores (Bass/Tile) — v2.

Sharding: row-partition of nodes, 8 ways. The dense input projection
(h = relu(x @ t1^T + b)) is replicated on every core into a 512B-stride
gather table [h bf16 x128 | b f32 | pad]. Per layer the edge phase is a
two-stream token walk ([all-lo windows][all-hi windows], int16 gather
indices split at lo_split): big dma_gather calls (6144 edges) fetch
source rows; per 128-edge tile a bf16 one-hot (4x DVE mode) both
recovers a[row] (tensor_tensor_reduce against a partition-replicated
a-broadcast) and scatter-adds w*h[col] into a per-window PSUM
accumulator via TensorE. Window results accumulate in SBUF across the
two streams. Between layers the owned rows are AllGathered. The head
(t2 matmul + log_softmax) runs as a final pass so the activation table
is not thrashed.
"""

import os
import sys
import numpy as np

sys.path.insert(0, "/opt/trn_rl_repo")

import concourse.bass as bass
import concourse.bacc as bacc
import concourse.mybir as mybir
import concourse.tile as tile
from concourse import library_config

F32 = mybir.dt.float32
BF16 = mybir.dt.bfloat16
I16 = mybir.dt.int16

# problem constants (self-contained per contract)
N_NODES = 50000
IN_CH = 256
HIDDEN = 128
OUT_CH = 64
EPS = 0.3
NCORES = 8
CALL_TOKENS = int(os.environ.get("KCT", "2048"))
CT_MAX = CALL_TOKENS // 128
EXT_SLOTS = 128   # 512B gather record
B_SLOT = 64       # f32 slot holding the gate b-term
PREP_GRP = 8


def _install_profile_hook():
    import types
    name = "antenv.axon_hooks"
    if name in sys.modules:
        return
    try:
        import trn_agent_boot.trn_boot as tb
        hook = tb._ntff_profile_via_ctypes("/opt/axon/libaxon_pjrt.so")
    except Exception:
        hook = None
    mod = types.ModuleType(name)
    mod._hook = hook
    mod.get_axon_ntff_profile_hook = lambda: mod._hook
    mod.set_axon_ntff_profile_hook = lambda h: setattr(mod, "_hook", h)
    sys.modules[name] = mod


# ======================================================================
# Host preprocessing: SPMD token streams + per-core data
# ======================================================================

def preprocess(edge_index, n_nodes, ncores, lo_split):
    row = np.asarray(edge_index[0], dtype=np.int64)
    col = np.asarray(edge_index[1], dtype=np.int64)
    E = row.shape[0]
    r_per = n_nodes // ncores
    nwin = (r_per + 127) // 128

    deg = np.bincount(row, minlength=n_nodes).astype(np.float64)
    dinv = np.where(deg > 0, 1.0 / np.sqrt(np.maximum(deg, 1.0)), 0.0)
    escale_all = (dinv[row] * dinv[col]).astype(np.float32)

    core = row // r_per
    lrow = row - core * r_per
    win = lrow // 128
    is_hi = (col >= lo_split).astype(np.int64)

    # stream order: core, then stream (lo/hi), then window, then lrow
    order = np.lexsort((lrow, win, is_hi, core))
    core_s, win_s, hi_s = core[order], win[order], is_hi[order]
    lrow_s, col_s, esc_s = lrow[order], col[order], escale_all[order]

    key = (core_s * 2 + hi_s) * nwin + win_s
    cnt = np.bincount(key, minlength=ncores * 2 * nwin).reshape(ncores, 2, nwin)
    sec_len = ((cnt.max(axis=0) + 127) // 128) * 128  # [2, nwin]
    L_lo = int(sec_len[0].sum())
    L_hi = int(sec_len[1].sum())
    e_tok = L_lo + L_hi
    sec_start = np.zeros((2, nwin), np.int64)
    sec_start[0] = np.concatenate([[0], np.cumsum(sec_len[0])[:-1]])
    sec_start[1] = L_lo + np.concatenate([[0], np.cumsum(sec_len[1])[:-1]])

    col16 = np.zeros((ncores, e_tok), np.int16)
    rowrel = np.zeros((ncores, e_tok), np.float32)
    esc = np.zeros((ncores, e_tok), np.float32)

    grp_first = np.zeros(ncores * 2 * nwin + 1, np.int64)
    np.cumsum(cnt.reshape(-1), out=grp_first[1:])
    rank = np.arange(E) - grp_first[key]
    dest = sec_start[hi_s, win_s] + rank
    cval = np.where(hi_s == 1, col_s - lo_split, col_s).astype(np.int16)
    col16[core_s, dest] = cval
    rowrel[core_s, dest] = (lrow_s - win_s * 128).astype(np.float32)
    esc[core_s, dest] = esc_s

    # gather calls per stream
    calls = []  # (stream, ts, nt)
    for h, base, L in ((0, 0, L_lo), (1, L_lo, L_hi)):
        off = 0
        while off < L:
            nt = min(CALL_TOKENS, L - off)
            calls.append((h, base + off, nt))
            off += nt

    idx_dev = np.zeros((ncores, 128, e_tok // 16), np.int16)
    for (h, ts, nt) in calls:
        blk = col16[:, ts:ts + nt].reshape(ncores, nt // 16, 16)
        blk = np.ascontiguousarray(np.transpose(blk, (0, 2, 1)))
        idx_dev[:, :, ts // 16:(ts + nt) // 16] = np.tile(blk, (1, 8, 1))
    rr_dev = np.ascontiguousarray(rowrel.reshape(ncores, -1, 128).transpose(0, 2, 1))
    es_dev = np.ascontiguousarray(esc.reshape(ncores, -1, 128).transpose(0, 2, 1))

    # transposed one-hot, partition-major: eht[r, i] = 1 iff rowrel[i] == r
    import ml_dtypes
    eht = np.zeros((ncores, 128 * e_tok), ml_dtypes.bfloat16)
    ii = np.arange(e_tok)
    for c in range(ncores):
        eht[c][rowrel[c].astype(np.int64) * e_tok + ii] = 1.0
    eht = eht.reshape(ncores, 128, e_tok)

    return {
        "nwin": nwin, "e_tok": e_tok, "sec_len": sec_len, "calls": calls,
        "idx_dev": idx_dev, "rr_dev": rr_dev, "es_dev": es_dev, "eht_dev": eht,
    }


# ======================================================================
# Kernel builder
# ======================================================================

def build_kernel(meta, n_nodes, in_ch, hidden, out_ch, eps, lo_split, ncores):
    nwin = meta["nwin"]
    e_tok = meta["e_tok"]
    sec_len = meta["sec_len"]
    calls = meta["calls"]
    nchunk_tot = (n_nodes + 127) // 128
    r_per = n_nodes // ncores
    last_win_rows = r_per - 128 * (nwin - 1)
    kt = in_ch // 128
    hh = hidden // 2  # f32 slots holding the bf16 h vector

    # tile -> window map, and burst boundaries per (stream, window)
    tiles_w = []
    burst = {}  # (h, w) -> (gfirst, glast) in global tile idx
    for h in range(2):
        for w in range(nwin):
            ntl = int(sec_len[h, w]) // 128
            if ntl == 0:
                continue
            g0 = len(tiles_w)
            tiles_w.extend([w] * ntl)
            burst[(h, w)] = (g0, g0 + ntl - 1)
    assert len(tiles_w) == e_tok // 128
    last_stream = {}
    for w in range(nwin):
        last_stream[w] = 1 if (1, w) in burst else 0

    ngrp = nchunk_tot // PREP_GRP
    grp_rem = nchunk_tot - ngrp * PREP_GRP
    ogrp = nwin // PREP_GRP
    ogrp_rem = nwin - ogrp * PREP_GRP

    nc = bacc.Bacc("TRN2", target_bir_lowering=False, debug=False,
                   num_devices=ncores, num_swdge_queues=4)

    # ---- I/O ----
    # xtg: host-prearranged [group, 128p, grp*kt, 128] bf16 (+ ones row separately)
    xtg = nc.dram_tensor("xtg", [ngrp + (1 if grp_rem else 0), 128, PREP_GRP * kt, 128], BF16, kind="ExternalInput")
    xbg = nc.dram_tensor("xbg", [ngrp + (1 if grp_rem else 0), 1, PREP_GRP, 128], BF16, kind="ExternalInput")
    xtog = nc.dram_tensor("xtog", [ogrp + (1 if ogrp_rem else 0), 128, PREP_GRP * kt, 128], BF16, kind="ExternalInput")
    xbog = nc.dram_tensor("xbog", [ogrp + (1 if ogrp_rem else 0), 1, PREP_GRP, 128], BF16, kind="ExternalInput")
    t1wt = nc.dram_tensor("t1wt", [in_ch + 1, hidden], BF16, kind="ExternalInput")
    gwrep = nc.dram_tensor("gwrep", [4, 128, hidden], BF16, kind="ExternalInput")
    gbrep = nc.dram_tensor("gbrep", [128, 2], F32, kind="ExternalInput")
    t2wt = nc.dram_tensor("t2wt", [hidden, out_ch], F32, kind="ExternalInput")
    t2b = nc.dram_tensor("t2b", [1, out_ch], F32, kind="ExternalInput")
    iota_in = nc.dram_tensor("iota", [128, 128], BF16, kind="ExternalInput")
    ident_in = nc.dram_tensor("ident", [128, 128], F32, kind="ExternalInput")
    ones_in = nc.dram_tensor("ones", [1, 128], F32, kind="ExternalInput")
    idx_in = nc.dram_tensor("idx", [128, e_tok // 16], I16, kind="ExternalInput")
    eht_in = nc.dram_tensor("eht", [128, e_tok], BF16, kind="ExternalInput")
    rr_in = nc.dram_tensor("rr", [128, e_tok // 128], F32, kind="ExternalInput")
    es_in = nc.dram_tensor("es", [128, e_tok // 128], F32, kind="ExternalInput")
    out = nc.dram_tensor("out", [r_per, out_ch], F32, kind="ExternalOutput")

    ext0 = nc.dram_tensor("ext0", [nchunk_tot * 128, EXT_SLOTS], F32)
    agi = nc.dram_tensor("agi", [r_per, EXT_SLOTS], F32)
    ago = nc.dram_tensor("ago", [r_per * ncores, EXT_SLOTS], F32)

    with tile.TileContext(nc) as tc:
        nc.gpsimd.load_library(library_config.mlp)
        with tc.tile_pool(name="consts", bufs=1) as cp:
            t1wt_sb = cp.tile([128, kt, hidden], BF16, tag="t1wt")
            nc.sync.dma_start(t1wt_sb[:], bass.AP(t1wt, 0, [[hidden, 128], [128 * hidden, kt], [1, hidden]]))
            t1b_sb = cp.tile([1, hidden], BF16, tag="t1b")
            nc.sync.dma_start(t1b_sb[:], t1wt.ap()[in_ch:in_ch + 1, :])
            gw_sb = cp.tile([128, 4, hidden], BF16, tag="gw")
            nc.sync.dma_start(gw_sb[:], bass.AP(gwrep, 0, [[hidden, 128], [128 * hidden, 4], [1, hidden]]))
            gb_sb = cp.tile([128, 2], F32, tag="gb")
            nc.sync.dma_start(gb_sb[:], gbrep.ap())
            t2wt_sb = cp.tile([128, out_ch], F32, tag="t2wt")
            nc.sync.dma_start(t2wt_sb[:], t2wt.ap())
            t2b_sb = cp.tile([1, out_ch], F32, tag="t2b")
            nc.sync.dma_start(t2b_sb[:], t2b.ap())
            iota_sb = cp.tile([128, 128], BF16, tag="iota")
            nc.sync.dma_start(iota_sb[:], iota_in.ap())
            ident_sb = cp.tile([128, 128], F32, tag="ident")
            nc.sync.dma_start(ident_sb[:], ident_in.ap())
            ones_sb = cp.tile([1, 128], F32, tag="ones")
            nc.sync.dma_start(ones_sb[:], ones_in.ap())
            idxt = cp.tile([128, e_tok // 16], I16, tag="idxt")
            nc.sync.dma_start(idxt[:], idx_in.ap())
            rr_sb = cp.tile([128, e_tok // 128], F32, tag="rr")
            nc.sync.dma_start(rr_sb[:], rr_in.ap())
            es_sb = cp.tile([128, e_tok // 128], F32, tag="es")
            nc.sync.dma_start(es_sb[:], es_in.ap())

            rawsc = cp.tile([128, nwin, hidden], F32, tag="rawsc")
            acc = cp.tile([128, nwin, hidden], F32, tag="acc")
            a_arr = cp.tile([128, nwin, 2], BF16, tag="a_arr")

            # ---------------- prep: replicated gather table ----------------
            with tc.tile_pool(name="prep", bufs=3) as pp, \
                 tc.tile_pool(name="prep_s", bufs=6) as pscr, \
                 tc.tile_pool(name="prep_ps", bufs=4, space="PSUM") as pps:

                def prep_group(gi, gcnt, xt_t, xb_t, own):
                    xt_sb = pp.tile([128, PREP_GRP * kt, 128], BF16, tag="xt")
                    nc.sync.dma_start(xt_sb[:, 0:gcnt * kt, :], xt_t.ap()[gi, :, 0:gcnt * kt, :])
                    xb_sb = pp.tile([1, PREP_GRP, 128], BF16, tag="xb")
                    nc.sync.dma_start(xb_sb[:, 0:gcnt, :], xb_t.ap()[gi, :, 0:gcnt, :])
                    if not own:
                        extg = pp.tile([128, PREP_GRP, B_SLOT + 1], F32, tag="extg")
                    for c in range(gcnt):
                        ps = pps.tile([128, hidden], F32, tag="h0ps")
                        for k in range(kt):
                            nc.tensor.matmul(ps[:], xt_sb[:, c * kt + k, :], t1wt_sb[:, k, :],
                                             start=(k == 0), stop=False)
                        nc.tensor.matmul(ps[:], xb_sb[:, c, :], t1b_sb[:], start=False, stop=True)
                        if own:
                            w = gi * PREP_GRP + c
                            nc.vector.tensor_scalar(out=rawsc[:, w, :], in0=ps[:],
                                                    scalar1=0.0, scalar2=eps,
                                                    op0=mybir.AluOpType.max,
                                                    op1=mybir.AluOpType.mult)
                            hb = pscr.tile([128, hidden], BF16, tag="hb")
                            nc.scalar.activation(hb[:], ps[:], mybir.ActivationFunctionType.Relu)
                            scr = pscr.tile([128, hidden], BF16, tag="scr")
                            a_f = pscr.tile([128, 1], F32, tag="af")
                            nc.vector.scalar_tensor_tensor(
                                out=scr[:], in0=hb[:], scalar=1.0, in1=gw_sb[:, 0, :],
                                op0=mybir.AluOpType.mult, op1=mybir.AluOpType.mult,
                                accum_out=a_f[:])
                            nc.vector.tensor_scalar(out=a_arr[:, w, 0:1], in0=a_f[:],
                                                    scalar1=gb_sb[:, 0:1], scalar2=None,
                                                    op0=mybir.AluOpType.add)
                        else:
                            hb = extg[:, c, 0:hh].bitcast(BF16)
                            nc.scalar.activation(hb, ps[:], mybir.ActivationFunctionType.Relu)
                            scr = pscr.tile([128, hidden], BF16, tag="scr")
                            nc.vector.scalar_tensor_tensor(
                                out=scr[:], in0=hb, scalar=1.0, in1=gw_sb[:, 1, :],
                                op0=mybir.AluOpType.mult, op1=mybir.AluOpType.mult,
                                accum_out=extg[:, c, B_SLOT:B_SLOT + 1])
                    if not own:
                        base = gi * PREP_GRP * 128
                        nc.sync.dma_start(
                            bass.AP(ext0, base * EXT_SLOTS,
                                    [[EXT_SLOTS, 128], [128 * EXT_SLOTS, gcnt], [1, B_SLOT + 1]]),
                            extg[:, 0:gcnt, :])

                for gi in range(ngrp + (1 if grp_rem else 0)):
                    prep_group(gi, PREP_GRP if gi < ngrp else grp_rem, xtg, xbg, False)
                for gi in range(ogrp + (1 if ogrp_rem else 0)):
                    prep_group(gi, PREP_GRP if gi < ogrp else ogrp_rem, xtog, xbog, True)

            # ---------------- edge phase (per layer) ----------------
            def emit_layer(l, table):
                lo_ap = table.ap()
                hi_ap = table.ap()[lo_split:, :]
                with tc.tile_pool(name=f"g{l}", bufs=int(os.environ.get("KGB", "2"))) as gp, \
                     tc.tile_pool(name=f"e{l}", bufs=int(os.environ.get("KEB", "2"))) as ep, \
                     tc.tile_pool(name=f"s{l}", bufs=3) as sp, \
                     tc.tile_pool(name=f"scr{l}", bufs=6) as scrp, \
                     tc.tile_pool(name=f"oh{l}", bufs=6) as ohp, \
                     tc.tile_pool(name=f"fin{l}", bufs=2) as fp, \
                     tc.tile_pool(name=f"psT{l}", bufs=3, space="PSUM") as psT, \
                     tc.tile_pool(name=f"psW{l}", bufs=2, space="PSUM") as psW:
                    def finalize(w):
                        rows = 128 if w < nwin - 1 else last_win_rows
                        if l == 0:
                            ext1 = fp.tile([128, B_SLOT + 1], F32, tag="ext1")
                            h1b = ext1[:, 0:hh].bitcast(BF16)
                            nc.vector.tensor_copy(h1b, acc[:, w, :])
                            scr = scrp.tile([128, hidden], BF16, tag="escr")
                            nc.vector.scalar_tensor_tensor(
                                out=scr[:], in0=h1b, scalar=1.0, in1=gw_sb[:, 3, :],
                                op0=mybir.AluOpType.mult, op1=mybir.AluOpType.mult,
                                accum_out=ext1[:, B_SLOT:B_SLOT + 1])
                            scr2 = scrp.tile([128, hidden], BF16, tag="escr2")
                            a_f = scrp.tile([128, 1], F32, tag="af1")
                            nc.vector.scalar_tensor_tensor(
                                out=scr2[:], in0=h1b, scalar=1.0, in1=gw_sb[:, 2, :],
                                op0=mybir.AluOpType.mult, op1=mybir.AluOpType.mult,
                                accum_out=a_f[:])
                            nc.vector.tensor_scalar(out=a_arr[:, w, 1:2], in0=a_f[:],
                                                    scalar1=gb_sb[:, 1:2], scalar2=None,
                                                    op0=mybir.AluOpType.add)
                            nc.sync.dma_start(agi.ap()[w * 128:w * 128 + rows, 0:B_SLOT + 1],
                                              ext1[0:rows, :])

                    # windows with no lo-burst: seed acc with rawsc; fully
                    # edgeless windows also finalize immediately
                    for w in range(nwin):
                        if (0, w) not in burst:
                            nc.vector.tensor_copy(acc[:, w, :], rawsc[:, w, :])
                            if (1, w) not in burst:
                                finalize(w)

                    qi = 0
                    W_ps = None
                    for (h, ts, nt) in [c for c in calls]:
                        ct = nt // 128
                        t0 = ts // 128
                        G = gp.tile([128, CT_MAX, EXT_SLOTS], F32, tag="G")
                        nc.gpsimd.dma_gather(
                            out_ap=G[:, 0:ct, :],
                            in_ap=(hi_ap if h else lo_ap),
                            idxs_ap=idxt[:, ts // 16:(ts + nt) // 16],
                            num_idxs=nt, num_idxs_reg=nt, elem_size=EXT_SLOTS,
                            single_packet=False, queue_num=qi % 4)
                        qi += 1
                        ehT = ep.tile([128, CT_MAX * 128], BF16, tag="ehT")
                        nc.sync.dma_start(
                            ehT[:, 0:ct * 128],
                            bass.AP(eht_in, t0 * 128, [[e_tok, 128], [1, ct * 128]]))
                        atokP = psT.tile([128, CT_MAX], F32, tag="atokP")
                        # split call into window-pure runs
                        runs = []
                        c = 0
                        while c < ct:
                            w = tiles_w[t0 + c]
                            c1 = c
                            while c1 < ct and tiles_w[t0 + c1] == w:
                                c1 += 1
                            runs.append((w, c, c1))
                            c = c1
                        # pass 1: gate argument for the whole call
                        for (w, c0, c1) in runs:
                            for c in range(c0, c1):
                                nc.tensor.matmul(atokP[:, c:c + 1],
                                                 ehT[:, c * 128:(c + 1) * 128],
                                                 a_arr[:, w, l:l + 1],
                                                 start=True, stop=True)
                        arg = sp.tile([128, CT_MAX], F32, tag="arg")
                        nc.vector.tensor_tensor(out=arg[:, 0:ct], in0=atokP[:, 0:ct],
                                                in1=G[:, 0:ct, B_SLOT],
                                                op=mybir.AluOpType.add)
                        gt = sp.tile([128, CT_MAX], F32, tag="gt")
                        nc.scalar.activation(gt[:, 0:ct], arg[:, 0:ct],
                                             mybir.ActivationFunctionType.Tanh)
                        wt = sp.tile([128, CT_MAX], F32, tag="wt")
                        nc.vector.tensor_tensor(out=wt[:, 0:ct], in0=gt[:, 0:ct],
                                                in1=es_sb[:, t0:t0 + ct],
                                                op=mybir.AluOpType.mult)
                        # pass 2: one-hot scatter matmuls
                        for (w, c0, c1) in runs:
                            bf, bl = burst[(h, w)]
                            if t0 + c0 == bf:
                                W_ps = psW.tile([128, hidden], F32, tag="W")
                            for c in range(c0, c1):
                                oh = ohp.tile([128, 128], BF16, tag="oh")
                                nc.vector.tensor_scalar(
                                    out=oh[:], in0=iota_sb[:],
                                    scalar1=rr_sb[:, t0 + c:t0 + c + 1],
                                    scalar2=wt[:, c:c + 1],
                                    op0=mybir.AluOpType.is_equal,
                                    op1=mybir.AluOpType.mult)
                                nc.tensor.matmul(W_ps[:], oh[:], G[:, c, 0:hh].bitcast(BF16),
                                                 start=(t0 + c == bf),
                                                 stop=(t0 + c == bl))
                            if t0 + c1 - 1 == bl:
                                if h == 0:
                                    nc.vector.tensor_tensor(out=acc[:, w, :], in0=W_ps[:],
                                                            in1=rawsc[:, w, :],
                                                            op=mybir.AluOpType.add)
                                    if last_stream[w] == 0:
                                        finalize(w)
                                else:
                                    nc.vector.tensor_tensor(out=acc[:, w, :], in0=W_ps[:],
                                                            in1=acc[:, w, :],
                                                            op=mybir.AluOpType.add)
                                    finalize(w)

            phase = os.environ.get("KPHASE", "head")
            plvl = {"prep": 0, "l0": 1, "cc": 2, "l1": 3, "head": 4}[phase]
            if plvl >= 1:
                emit_layer(0, ext0)
            if plvl >= 2:
                nc.gpsimd.collective_compute(
                    "AllGather", mybir.AluOpType.bypass,
                    replica_groups=[list(range(ncores))],
                    ins=[agi.ap().opt()], outs=[ago.ap().opt()])
            if plvl >= 3:
                emit_layer(1, ago)
            if plvl < 4:
                with tc.tile_pool(name="zout", bufs=1) as zp:
                    o_z = zp.tile([128, out_ch], F32, tag="oz")
                    nc.vector.memset(o_z[:], 0.0)
                    for w in range(nwin):
                        rows = 128 if w < nwin - 1 else last_win_rows
                        nc.sync.dma_start(out.ap()[w * 128:w * 128 + rows, :],
                                          o_z[0:rows, :])
                return nc

            # ---------------- head: out = log_softmax(h @ t2^T + b) ----------
            # two passes so the Act engine loads the Exp/Ln tables once each
            with tc.tile_pool(name="head", bufs=4) as hp, \
                 tc.tile_pool(name="head_ps", bufs=4, space="PSUM") as hps:
                o_all = cp.tile([128, nwin, out_ch], F32, tag="o_all")
                nm_all = cp.tile([128, nwin], F32, tag="nm_all")
                s_all = cp.tile([128, nwin], F32, tag="s_all")
                for w in range(nwin):
                    ht_ps = hps.tile([128, 128], F32, tag="ht")
                    nc.tensor.matmul(ht_ps[:], acc[:, w, :], ident_sb[:],
                                     start=True, stop=True)
                    ht_sb = hp.tile([128, 128], F32, tag="ht_sb")
                    nc.vector.tensor_copy(ht_sb[:], ht_ps[:])
                    o_ps = hps.tile([128, out_ch], F32, tag="ops")
                    nc.tensor.matmul(o_ps[:], ht_sb[:], t2wt_sb[:], start=True, stop=False)
                    nc.tensor.matmul(o_ps[:], ones_sb[:], t2b_sb[:], start=False, stop=True)
                    nc.vector.reduce_max(out=nm_all[:, w:w + 1], in_=o_ps[:],
                                         axis=mybir.AxisListType.X, negate=True)
                    e_sb = hp.tile([128, out_ch], F32, tag="e")
                    nc.scalar.activation(e_sb[:], o_ps[:],
                                         mybir.ActivationFunctionType.Exp,
                                         bias=nm_all[:, w:w + 1])
                    nc.vector.reduce_sum(out=s_all[:, w:w + 1], in_=e_sb[:],
                                         axis=mybir.AxisListType.X)
                    nc.vector.tensor_copy(o_all[:, w, :], o_ps[:])
                ls_all = cp.tile([128, nwin], F32, tag="ls_all")
                nc.scalar.activation(ls_all[:], s_all[:], mybir.ActivationFunctionType.Ln)
                for w in range(nwin):
                    rows = 128 if w < nwin - 1 else last_win_rows
                    o_sb = hp.tile([128, out_ch], F32, tag="o")
                    nc.vector.tensor_scalar(out=o_sb[:], in0=o_all[:, w, :],
                                            scalar1=nm_all[:, w:w + 1],
                                            scalar2=ls_all[:, w:w + 1],
                                            op0=mybir.AluOpType.add,
                                            op1=mybir.AluOpType.subtract)
                    nc.sync.dma_start(out.ap()[w * 128:w * 128 + rows, :], o_sb[0:rows, :])

    return nc


# ======================================================================
# Host driver
# ======================================================================

def _bf16(a):
    import ml_dtypes
    return np.asarray(a, dtype=ml_dtypes.bfloat16)


def _group_x(xT_pad, nrow_units, kt):
    # xT_pad: [in_ch+1, units*128] f32 -> xtg [ngrp, 128, PREP_GRP*kt, 128],
    # xbg [ngrp, 1, PREP_GRP, 128] (ones row)
    in_ch = (xT_pad.shape[0] - 1)
    ngrp_t = (nrow_units + PREP_GRP - 1) // PREP_GRP
    pad_units = ngrp_t * PREP_GRP
    xp = np.zeros((in_ch + 1, pad_units * 128), np.float32)
    xp[:, :xT_pad.shape[1]] = xT_pad
    # [in, u, 128] -> [u, in, 128]
    xr = xp[:in_ch].reshape(in_ch, pad_units, 128).transpose(1, 0, 2)
    # [g, c, k, p, r] with in = k*128+p
    xg = xr.reshape(ngrp_t, PREP_GRP, kt, 128, 128)
    xtg = np.ascontiguousarray(xg.transpose(0, 3, 1, 2, 4)).reshape(
        ngrp_t, 128, PREP_GRP * kt, 128)
    xb = xp[in_ch].reshape(ngrp_t, 1, PREP_GRP, 128)
    return _bf16(xtg), _bf16(np.ascontiguousarray(xb))


def kernel_run(x, edge_index, t1_w, t1_b, gate_w, gate_b, t2_w, t2_b,
               n_nodes=N_NODES, in_ch=IN_CH, hidden=HIDDEN, out_ch=OUT_CH,
               eps=EPS, ncores=NCORES, lo_split=None, trace=False):
    _install_profile_hook()
    from concourse import bass_utils

    if lo_split is None:
        lo_split = min(25000, ((n_nodes + 1) // 2 + 127) // 128 * 128)
    meta = preprocess(edge_index, n_nodes, ncores, lo_split)
    nwin = meta["nwin"]
    r_per = n_nodes // ncores
    nchunk_tot = (n_nodes + 127) // 128
    kt = in_ch // 128

    nc = build_kernel(meta, n_nodes, in_ch, hidden, out_ch, eps, lo_split, ncores)
    nc.finalize()

    # host arrays
    x = np.asarray(x, np.float32)
    xT = np.concatenate([x.T, np.ones((1, x.shape[0]), np.float32)], axis=0)  # [in+1, N]
    pad_n = nchunk_tot * 128
    xT_pad = np.zeros((in_ch + 1, pad_n), np.float32)
    xT_pad[:, :n_nodes] = xT
    xtg_h, xbg_h = _group_x(xT_pad, nchunk_tot, kt)

    t1wt_h = _bf16(np.concatenate([np.asarray(t1_w, np.float32).T,
                                   np.asarray(t1_b, np.float32)[None, :]], axis=0))
    gw = np.asarray(gate_w, np.float32)
    gwrep_h = _bf16(np.stack([
        np.tile(gw[0, :hidden][None, :], (128, 1)),
        np.tile(gw[0, hidden:][None, :], (128, 1)),
        np.tile(gw[1, :hidden][None, :], (128, 1)),
        np.tile(gw[1, hidden:][None, :], (128, 1))]))
    gbrep_h = np.tile(np.asarray(gate_b, np.float32)[None, :], (128, 1))
    t2wt_h = np.ascontiguousarray(np.asarray(t2_w, np.float32).T)
    t2b_h = np.asarray(t2_b, np.float32)[None, :]
    iota_h = _bf16(np.tile(np.arange(128, dtype=np.float32)[None, :], (128, 1)))
    ident_h = np.eye(128, dtype=np.float32)
    ones_h = np.ones((1, 128), np.float32)

    in_maps = []
    for c in range(ncores):
        sl = np.zeros((in_ch + 1, nwin * 128), np.float32)
        take = min(nwin * 128, xT.shape[1] - c * r_per)
        sl[:, :take] = xT[:, c * r_per: c * r_per + take]
        xtog_h, xbog_h = _group_x(sl, nwin, kt)
        in_maps.append({
            "xtg": xtg_h, "xbg": xbg_h, "xtog": xtog_h, "xbog": xbog_h,
            "t1wt": t1wt_h, "gwrep": gwrep_h, "gbrep": gbrep_h,
            "t2wt": t2wt_h, "t2b": t2b_h,
            "iota": iota_h, "ident": ident_h, "ones": ones_h,
            "idx": meta["idx_dev"][c], "rr": meta["rr_dev"][c],
            "es": meta["es_dev"][c], "eht": meta["eht_dev"][c],
        })

    res = bass_utils.run_bass_kernel_spmd(
        nc, in_maps, core_ids=list(range(ncores)), trace=trace)
    outp = np.concatenate([res.results[c]["out"] for c in range(ncores)], axis=0)
    return outp[:n_nodes], res


def kernel(**inputs):
    x = inputs["x"]
    edge_index = inputs["edge_index"]
    outp, _ = kernel_run(
        x, edge_index, inputs["t1_w"], inputs["t1_b"], inputs["gate_w"],
        inputs["gate_b"], inputs["t2_w"], inputs["t2_b"])
    return np.asarray(outp, np.float32)
